# revision 1
# baseline (speedup 1.0000x reference)
"""Concordance-index (C-index) kernel for Trainium2, 8 NeuronCores — v3.

Math
----
Reference computes, over all pairs i<j of N=16384 samples:
    cc = ((y_i>=y_j & yh_i>=yh_j & st_j) | (y_i<=y_j & yh_i<=yh_j & st_i)) & triu
    tp = ((y_i<=y_j & st_i) | (y_i>=y_j & st_j)) & triu
    out = sum(cc) / sum(tp)

Columns with st_j = 0 contribute nothing, so the sweep is N x ns over
(all i) x (event j): with A = [y_i >= y_j], B = [yh_i >= yh_j],
    sum(cc) = S1 - ns,  S1 = sum_{i, j in E} A*B   (diag = 1 each)
    sum(tp) = S2 - ns,  S2 = sum_{i, j in E} A

v3 key idea: the host permutes the i-axis to y-sorted order and packs
event columns y-sorted + rank-interleaved across cores.  Then per
128-event j-group, A is a step function along i, and most [128, 4096]
i-tiles are fully decided:
    "zero" tile (all y_i < all y_j): contributes nothing -> SKIPPED
    "ones" tile (all y_i >= all y_j): A==1, so only sum(B) is needed ->
        one DVE ts-accum (or ScalarE sign-accum) per tile
    "mixed" tile (~1.3 per group): full compute, flavored A (DVE
        ts-plain a01 + stt fused b*a with row-accum; PE counts a01) or
        B (2 ScalarE Signs + DVE tt product + PE ones-matmul) to
        balance DVE/ACT/PE.
This cuts tile-ops from 32 full to ~11 mixed + ~9 cheap per core.
The (pattern, flavors) are data-dependent -> compiled per pattern and
cached. Pads use y_j=-BIG / yh_j=+BIG so every formula contributes an
exact host-known constant. bf16 tie noise ~5e-4 total, gate is 2e-2.
"""

import math
import os
import sys

import numpy as np

for _p in ("/opt/trn_rl_repo", "/root/.axon_site", "/root/.axon_site/_ro/trn_rl_repo"):
    if os.path.isdir(_p) and _p not in sys.path:
        sys.path.append(_p)

import ml_dtypes

import concourse.bacc as bacc
import concourse.mybir as mybir
from concourse import bass_utils
from concourse import tile

N = 16384
P = 128
NCORES = 8
F = 4096                 # i-tile width (free axis)
IT = N // F              # 4 i-tiles
BIG = np.float32(1e30)

FP32 = mybir.dt.float32
BF16 = mybir.dt.bfloat16
Alu = mybir.AluOpType
ActF = mybir.ActivationFunctionType

# tile classes
ZERO, ONES_DVE, ONES_ACT, MIX_A, MIX_B = 0, 1, 2, 3, 4


def _bf(x):
    return np.asarray(x, np.float32).astype(ml_dtypes.bfloat16).astype(np.float32)


class Shard:
    """Host-side packing: y-sorted i-axis, rank-interleaved event slots,
    per-(group, itile) class pattern."""

    def __init__(self, y, yh, status):
        y = np.asarray(y, np.float32)
        yh = np.asarray(yh, np.float32)
        ybf = _bf(y)
        yhbf = _bf(yh)
        # i-axis permutation: stable sort by bf16 y
        self.idx = np.argsort(ybf, kind="stable")
        self.y_sorted = ybf[self.idx]          # fed as y_full
        self.yh_perm = yhbf[self.idx]          # fed as yh_full
        pos_of_orig = np.empty(N, np.int64)
        pos_of_orig[self.idx] = np.arange(N)

        ev = np.nonzero(np.asarray(status) == 1)[0]
        self.ns = len(ev)
        # events sorted by bf16 y, rank r -> core r % NCORES
        ev_sorted = ev[np.argsort(ybf[ev], kind="stable")]
        self.jt_e = max(1, math.ceil(self.ns / (NCORES * P)))
        per = self.jt_e * P
        self.per = per
        self.nt = IT * self.jt_e

        # slot tables per core: orig index (-1 = pad), y/yh scalar values
        self.slot_orig = np.full((NCORES, per), -1, np.int64)
        self.y_sl = np.full((NCORES, per), -BIG, np.float32)
        self.yh_sl = np.full((NCORES, per), BIG, np.float32)
        for c in range(NCORES):
            mine = ev_sorted[c::NCORES]
            k = len(mine)
            self.slot_orig[c, :k] = mine
            self.y_sl[c, :k] = ybf[mine]
            self.yh_sl[c, :k] = yhbf[mine]

        # crossing positions: first sorted-i with y_i >= y_j
        # (exact: both sides bf16 values)
        ssl = np.searchsorted(self.y_sorted, self.y_sl.reshape(-1),
                              side="left").reshape(NCORES, per)

        # per-(group, itile) class, unioned across cores
        cls = np.zeros((self.jt_e, IT), np.int64)
        self.group_lo = np.zeros((NCORES, self.jt_e), np.int64)
        self.group_hi = np.zeros((NCORES, self.jt_e), np.int64)
        for g in range(self.jt_e):
            lo_all, hi_all = N, 0
            for c in range(NCORES):
                sl = slice(g * P, (g + 1) * P)
                real = self.slot_orig[c, sl] >= 0
                if real.any():
                    lo = int(ssl[c, sl][real].min())
                    hi = int(ssl[c, sl][real].max())
                else:
                    lo, hi = N, 0   # all-pad group: everything "zero"
                self.group_lo[c, g] = lo
                self.group_hi[c, g] = hi
                lo_all = min(lo_all, lo)
                hi_all = max(hi_all, hi)
            for it in range(IT):
                t0, t1 = it * F, (it + 1) * F
                if t1 <= lo_all:
                    cls[g, it] = ZERO
                elif t0 >= hi_all:
                    cls[g, it] = ONES_DVE
                else:
                    cls[g, it] = MIX_A
        self.cls = cls
        self._assign_flavors()
        # event diag position (sorted-i space) per core/slot
        self.pos_of_orig = pos_of_orig
        self.pattern = tuple(cls.reshape(-1).tolist())

    def _assign_flavors(self):
        """Balance DVE / ACT / PE by greedy assignment (measured ns/tile)."""
        cost = {MIX_A: (6000, 0, 5200), MIX_B: (2600, 9900, 5200),
                ONES_DVE: (4800, 0, 0), ONES_ACT: (0, 4980, 0)}
        load = [0.0, 0.0, 0.0]
        order = [(g, it) for g in range(self.jt_e) for it in range(IT)]
        # mixed first (larger), then ones
        for kind, opts in ((MIX_A, (MIX_A, MIX_B)),
                           (ONES_DVE, (ONES_DVE, ONES_ACT))):
            for g, it in order:
                if self.cls[g, it] != kind:
                    continue
                best, bestmax = None, None
                for o in opts:
                    trial = [load[k] + cost[o][k] for k in range(3)]
                    m = max(trial)
                    if bestmax is None or m < bestmax:
                        bestmax, best = m, o
                self.cls[g, it] = best
                for k in range(3):
                    load[k] += cost[best][k]
        self.load = load


def build_bass(shard):
    jt_e, nt, cls = shard.jt_e, shard.nt, shard.cls
    nc = bacc.Bacc(debug=False, num_devices=NCORES)

    y_full = nc.dram_tensor("y_full", [1, N], BF16, kind="ExternalInput")
    yh_full = nc.dram_tensor("yh_full", [1, N], BF16, kind="ExternalInput")
    y_sl = nc.dram_tensor("y_sl", [P, jt_e], FP32, kind="ExternalInput")
    yh_sl = nc.dram_tensor("yh_sl", [P, jt_e], FP32, kind="ExternalInput")
    o_r = nc.dram_tensor("o_r", [P, nt], FP32, kind="ExternalOutput")
    o_g = nc.dram_tensor("o_g", [P, nt], FP32, kind="ExternalOutput")
    o_h = nc.dram_tensor("o_h", [P, nt], FP32, kind="ExternalOutput")
    o_b = nc.dram_tensor("o_b", [P, nt], FP32, kind="ExternalOutput")
    o_pa = nc.dram_tensor("o_pa", [1, 512], FP32, kind="ExternalOutput")
    o_pp = nc.dram_tensor("o_pp", [1, 512], FP32, kind="ExternalOutput")

    n_mm = {"pa": int((cls == MIX_A).sum()) * (F // 512),
            "pp": int((cls == MIX_B).sum()) * (F // 512)}
    # which i-tiles are needed at all
    it_used = [it for it in range(IT)
               if any(cls[g, it] != ZERO for g in range(jt_e))]

    with tile.TileContext(nc) as tc:
        with (
            tc.tile_pool(name="const", bufs=1) as cpool,
            tc.tile_pool(name="bcast", bufs=2) as bpool,
            tc.tile_pool(name="work", bufs=3) as wpool,
            tc.tile_pool(name="psum", bufs=1, space="PSUM") as ppool,
        ):
            y_j = cpool.tile([P, jt_e], FP32)
            nc.sync.dma_start(out=y_j[:, :], in_=y_sl[:, :])
            yh_j = cpool.tile([P, jt_e], FP32)
            nc.sync.dma_start(out=yh_j[:, :], in_=yh_sl[:, :])
            neg_y = cpool.tile([P, jt_e], FP32)
            nc.vector.tensor_scalar_mul(neg_y[:, :], y_j[:, :], -1.0)
            neg_yh = cpool.tile([P, jt_e], FP32)
            nc.vector.tensor_scalar_mul(neg_yh[:, :], yh_j[:, :], -1.0)

            ones_w = cpool.tile([P, 1], BF16)
            nc.vector.memset(ones_w[:, :], 1.0)

            accs = {}
            for nm in ("r", "g", "h", "b"):
                # no memset: combine() only reads columns their class's
                # accum op writes (accum_out overwrites, not adds)
                t = cpool.tile([P, nt], FP32, tag=f"acc_{nm}")
                accs[nm] = t
            acc_pa = ppool.tile([1, 512], FP32)
            acc_pp = ppool.tile([1, 512], FP32)
            seen = {"pa": 0, "pp": 0}

            def pe_reduce(key, acc, src):
                for ch in range(F // 512):
                    seen[key] += 1
                    nc.tensor.matmul(
                        acc[0:1, 0:512],
                        ones_w[:, :],
                        src[:, ch * 512:(ch + 1) * 512],
                        start=(seen[key] == 1),
                        stop=(seen[key] == n_mm[key]),
                    )

            for it in it_used:
                need_y = any(cls[g, it] in (MIX_A, MIX_B) for g in range(jt_e))
                yib = None
                if need_y:
                    yib = bpool.tile([P, F], BF16, tag="yib")
                    nc.sync.dma_start(
                        out=yib[:, :],
                        in_=y_full[0:1, it * F:(it + 1) * F].to_broadcast((P, F)),
                    )
                yhib = bpool.tile([P, F], BF16, tag="yhib")
                nc.sync.dma_start(
                    out=yhib[:, :],
                    in_=yh_full[0:1, it * F:(it + 1) * F].to_broadcast((P, F)),
                )
                for g in range(jt_e):
                    col = it * jt_e + g
                    k = cls[g, it]
                    if k == ZERO:
                        continue
                    if k == ONES_DVE:
                        b01 = wpool.tile([P, F], BF16, tag="stt_out")
                        nc.vector.tensor_scalar(
                            out=b01[:, :], in0=yhib[:, :],
                            scalar1=yh_j[:, g:g + 1], scalar2=0.0,
                            op0=Alu.is_ge, op1=Alu.add,
                            accum_out=accs["b"][:, col:col + 1],
                        )
                    elif k == ONES_ACT:
                        hs = wpool.tile([P, F], BF16, tag="hs")
                        nc.scalar.activation(
                            out=hs[:, :], in_=yhib[:, :], func=ActF.Sign,
                            bias=neg_yh[:, g:g + 1], scale=1.0,
                            accum_out=accs["h"][:, col:col + 1],
                        )
                    elif k == MIX_A:
                        a01 = wpool.tile([P, F], BF16, tag="a01")
                        nc.vector.tensor_scalar(
                            out=a01[:, :], in0=yib[:, :],
                            scalar1=y_j[:, g:g + 1], scalar2=None,
                            op0=Alu.is_ge,
                        )
                        pab = wpool.tile([P, F], BF16, tag="stt_out")
                        nc.vector.scalar_tensor_tensor(
                            out=pab[:, :], in0=yhib[:, :],
                            scalar=yh_j[:, g:g + 1], in1=a01[:, :],
                            op0=Alu.is_ge, op1=Alu.mult,
                            accum_out=accs["r"][:, col:col + 1],
                        )
                        pe_reduce("pa", acc_pa, a01)
                    else:  # MIX_B
                        gs = wpool.tile([P, F], BF16, tag="gs")
                        nc.scalar.activation(
                            out=gs[:, :], in_=yib[:, :], func=ActF.Sign,
                            bias=neg_y[:, g:g + 1], scale=1.0,
                            accum_out=accs["g"][:, col:col + 1],
                        )
                        hs = wpool.tile([P, F], BF16, tag="hs")
                        nc.scalar.activation(
                            out=hs[:, :], in_=yhib[:, :], func=ActF.Sign,
                            bias=neg_yh[:, g:g + 1], scale=1.0,
                            accum_out=accs["h"][:, col:col + 1],
                        )
                        p = wpool.tile([P, F], BF16, tag="p")
                        nc.vector.tensor_tensor(
                            out=p[:, :], in0=gs[:, :], in1=hs[:, :],
                            op=Alu.mult)
                        pe_reduce("pp", acc_pp, p)

            nc.sync.dma_start(out=o_r[:, :], in_=accs["r"][:, :])
            nc.sync.dma_start(out=o_g[:, :], in_=accs["g"][:, :])
            nc.sync.dma_start(out=o_h[:, :], in_=accs["h"][:, :])
            nc.sync.dma_start(out=o_b[:, :], in_=accs["b"][:, :])
            for acc, o, key in ((acc_pa, o_pa, "pa"), (acc_pp, o_pp, "pp")):
                stg = cpool.tile([1, 512], FP32, tag=f"stg_{o.name}")
                if n_mm[key] == 0:
                    nc.vector.memset(stg[:, :], 0.0)
                else:
                    nc.vector.tensor_copy(out=stg[:, :], in_=acc[0:1, 0:512])
                nc.sync.dma_start(out=o[:, :], in_=stg[:, :])

    nc.compile()
    return nc


_NC_CACHE = {}


def _get_nc(shard):
    key = (shard.jt_e, shard.pattern)
    if key not in _NC_CACHE:
        _NC_CACHE[key] = build_bass(shard)
    return _NC_CACHE[key]


def make_in_maps(shard):
    y2 = np.ascontiguousarray(
        shard.y_sorted.astype(ml_dtypes.bfloat16).reshape(1, N))
    yh2 = np.ascontiguousarray(
        shard.yh_perm.astype(ml_dtypes.bfloat16).reshape(1, N))
    in_maps = []
    for c in range(NCORES):
        in_maps.append({
            "y_full": y2,
            "yh_full": yh2,
            # slot s = g*P + p  ->  [p, g]
            "y_sl": np.ascontiguousarray(
                shard.y_sl[c].reshape(shard.jt_e, P).T),
            "yh_sl": np.ascontiguousarray(
                shard.yh_sl[c].reshape(shard.jt_e, P).T),
        })
    return in_maps


def combine(results, shard):
    """Exact reconstruction in float64 from device partial sums."""
    jt_e, nt, cls = shard.jt_e, shard.nt, shard.cls
    Ff = float(F)
    S1 = 0.0
    S2 = 0.0
    n_pad = (shard.slot_orig < 0).sum(axis=1)  # per core (in last group)
    for c, r in enumerate(results):
        rr = r["o_r"].astype(np.float64)
        gg = r["o_g"].astype(np.float64)
        hh = r["o_h"].astype(np.float64)
        bb = r["o_b"].astype(np.float64)
        pa = float(r["o_pa"].astype(np.float64).sum())
        pp = float(r["o_pp"].astype(np.float64).sum())
        real = (shard.slot_orig[c] >= 0).reshape(jt_e, P)  # [g, p]
        S2 += pa  # pad pollution removed below
        for g in range(jt_e):
            nreal = int(real[g].sum())
            npad = P - nreal
            for it in range(IT):
                col = it * jt_e + g
                k = cls[g, it]
                if k == ZERO:
                    continue
                if k == ONES_DVE:
                    # A==1 for real slots: S1 += sum(B), S2 += F per real
                    S1 += float(bb[:, col][real[g]].sum())
                    S2 += Ff * nreal
                elif k == ONES_ACT:
                    # sum(B) ~= (F + sum(h_sign))/2 per real partition
                    S1 += float(
                        (Ff + hh[:, col][real[g]]).sum()) / 2.0
                    S2 += Ff * nreal
                elif k == MIX_A:
                    S1 += float(rr[:, col].sum())   # pads contribute 0
                    S2 -= Ff * npad                  # pad a01==1 rows in pa
                else:  # MIX_B
                    # per-cell (1+g)(1+h)/4 identity: pads cancel exactly
                    S1 += (Ff * P + float(gg[:, col].sum())
                           + float(hh[:, col].sum())) / 4.0
                    # (F + sum g)/2 per partition; pad rows give F -> remove
                    S2 += (Ff * P + float(gg[:, col].sum())) / 2.0 - Ff * npad
        # gh product term of the MIX_B tiles (PSUM-accumulated per core)
        S1 += pp / 4.0
    # diagonal corrections for MIX_B tiles (est 1/4 resp 1/2, want 1)
    for c in range(NCORES):
        for s in range(shard.per):
            o = shard.slot_orig[c, s]
            if o < 0:
                continue
            g = s // P
            it = int(shard.pos_of_orig[o]) // F
            if cls[g, it] == MIX_B:
                S1 += 0.75
                S2 += 0.5
    ns = float(shard.ns)
    c32 = np.float32(S1 - ns)
    t32 = np.float32(S2 - ns)
    return np.asarray(np.float32(c32 / t32))


def kernel(y, y_hat, status, _run_kwargs=None):
    shard = Shard(y, y_hat, status)
    nc = _get_nc(shard)
    in_maps = make_in_maps(shard)
    kw = dict(_run_kwargs or {})
    res = bass_utils.run_bass_kernel_spmd(
        nc, in_maps, core_ids=list(range(NCORES)), **kw)
    out = combine(res.results, shard)
    if _run_kwargs is not None:
        return out, res
    return out


if __name__ == "__main__":
    rng = np.random.default_rng(0)
    y = rng.standard_normal(N).astype(np.float32)
    yh = rng.standard_normal(N).astype(np.float32)
    st = (rng.integers(0, 2, N)).astype(np.int32)
    print(kernel(y, yh, st))



# revision 12
# speedup vs baseline: 1.6598x; 1.6598x over previous
"""Concordance-index (C-index) kernel for Trainium2, 8 NeuronCores — v4.

Math
----
Reference computes, over all pairs i<j of N=16384 samples:
    cc = ((y_i>=y_j & yh_i>=yh_j & st_j) | (y_i<=y_j & yh_i<=yh_j & st_i)) & triu
    tp = ((y_i<=y_j & st_i) | (y_i>=y_j & st_j)) & triu
    out = sum(cc) / sum(tp)

With A_ae = [y_a >= y_e], B_ae = [yh_a >= yh_e] over (a = all, e = event):
    sum(cc) = S1 - ns - OV,   S1 = sum_e T_e,  T_e = sum_a A_ae * B_ae
    sum(tp) = S2 - ns - OV2,  S2 = sum_e sum_a A_ae
where OV / OV2 are exact-fp32-tie corrections among event pairs.

v4 key ideas
------------
* Sort the a-axis by fp32 y. Then A_ae = [a >= c_e] with c_e =
  searchsorted(y_sorted, y_e) EXACT on the host: S2 is pure host math,
  and T_e = count of [yh_a >= t_e] over the suffix [c_e, N) — or, via
  the host-known total R_e = |{a: yh_a >= t_e}|, R_e minus the prefix
  count over [0, c_e).  Each event only pays its SHORTER side:
  sum_e min(c_e, N-c_e) ~ N^2/8 compares instead of N^2/2.
* Events sorted by c, 1024 per row (128/core x 8 cores).  Per row the
  ragged boundary band [lo_r, hi_r] is absorbed by a HOST-GATHERED
  per-event window (padded with -60000 => compares give 0), the rest is
  a row-uniform tail range — so EVERY device op is a plain fused
  compare(+accumulate) on [128, W] tiles: no masks, no products.
* S1 only needs GLOBAL sums, so reductions may go over any axis: three
  engine lanes run concurrently — ACT (Sign+accum, 0.95 ns/elem), DVE
  (is_ge+accum fused, 1.09), and DVE plain is_ge at 4x DVE perf mode
  (0.29) + PE ones-matmul column-reduce into PSUM (1.08).
* fp16 compare semantics on host and device alike; only fp16-vs-fp32
  tie noise remains (~1e-4 rel; gate is 2e-2).
"""

import math
import os
import sys

import numpy as np

for _p in ("/opt/trn_rl_repo", "/root/.axon_site", "/root/.axon_site/_ro/trn_rl_repo"):
    if os.path.isdir(_p) and _p not in sys.path:
        sys.path.append(_p)

import ml_dtypes  # noqa: F401  (env check)

import concourse.bacc as bacc
import concourse.mybir as mybir
from concourse import bass_utils
from concourse import tile

N = 16384
P = 128
NCORES = 8
NEG16 = np.float16(-60000.0)
BIG32 = np.float32(1e30)
TCH = 3072          # tail chunk width for lane granularity
MM = 512            # PE matmul moving width (one PSUM bank)

FP32 = mybir.dt.float32
FP16 = mybir.dt.float16
BF16 = mybir.dt.bfloat16
Alu = mybir.AluOpType
ActF = mybir.ActivationFunctionType

# measured ns/elem and ns/op overhead (TRN2 microbench, [128, 4096] ops)
COST = {
    "dve": (1.09, 220.0),    # tensor_scalar is_ge+add with accum (1x)
    "act": (0.95, 300.0),    # activation Sign with accum
    "pe": (0.29, 130.0),     # DVE plain is_ge at 4x; PE adds 1.08/elem
}
PE_RATE = 1.08               # ns/elem for ones-matmul reduce (incl ldweights)


def _mid32(t16):
    """fp32 midpoint between t16 and the next-smaller fp16 value.
    sign(x - mid) = +1  <=>  x >= t16   for every fp16 x."""
    prev = np.nextafter(t16, np.float16(-np.inf))
    return (t16.astype(np.float32) + prev.astype(np.float32)) * np.float32(0.5)


class Plan:
    def __init__(self, y, yh, status):
        y32 = np.asarray(y, np.float32)
        yh32 = np.asarray(yh, np.float32)
        st = np.asarray(status)

        order = np.argsort(y32, kind="stable")
        ys = y32[order]
        self.yh16 = yh32[order].astype(np.float16)

        ev = np.nonzero(st == 1)[0]
        self.ns = int(len(ev))
        t16 = yh32[ev].astype(np.float16)
        c = np.searchsorted(ys, y32[ev], side="left").astype(np.int64)
        yh16s = np.sort(self.yh16)
        R = (N - np.searchsorted(yh16s, t16, side="left")).astype(np.int64)
        self.S2 = int((np.int64(N) - c).sum())

        yv = y32[ev]
        _, cnt = np.unique(yv, return_counts=True)
        self.OV2 = int((cnt.astype(np.int64) * (cnt - 1) // 2).sum())
        pair = np.stack([yv, yh32[ev]], 1)
        _, cnt2 = np.unique(pair, axis=0, return_counts=True)
        self.OV = int((cnt2.astype(np.int64) * (cnt2 - 1) // 2).sum())

        eo = np.argsort(c, kind="stable")
        self.c = c[eo]
        self.t16 = t16[eo]
        self.R = R[eo]

        self.nrow = max(1, math.ceil(self.ns / (P * NCORES)))
        self.rows = []
        for r in range(self.nrow):
            s, e = r * P * NCORES, min((r + 1) * P * NCORES, self.ns)
            cr = self.c[s:e]
            lo, hi = int(cr.min()), int(cr.max())
            suffix = int((np.int64(N) - cr).sum()) <= int(cr.sum())
            if suffix:
                wh = hi + 1 - lo      # head window [c_e, hi+1)
                tail = (hi + 1, N)    # uniform tail
            else:
                wh = hi - lo          # head window [lo, c_e)
                tail = (0, lo)
            wh = max(wh, 1)
            wh = ((wh + MM - 1) // MM) * MM  # 512-pad for the PE lane
            self.rows.append({"suffix": suffix, "s": s, "e": e,
                              "lo": lo, "hi": hi, "wh": wh, "tail": tail})

        # ---- op list: per row one head + tail chunks ----
        self.ops = []   # (kind, row, (start, end) or None)
        for r, row in enumerate(self.rows):
            self.ops.append(("head", r, None))
            t0, t1 = row["tail"]
            x = t0
            while x < t1:
                self.ops.append(("tail", r, (x, min(x + TCH, t1))))
                x += TCH

        def op_elems(i):
            kind, r, x = self.ops[i]
            return self.rows[r]["wh"] if kind == "head" else x[1] - x[0]

        # ---- lane assignment: greedy 3-lane makespan ----
        load = {"dve": 0.0, "act": 0.0, "pe": 0.0}
        self.lane = [None] * len(self.ops)
        for i in sorted(range(len(self.ops)), key=lambda i: -op_elems(i)):
            el = op_elems(i)
            # pe lane only for widths that are 512 multiples
            cands = ["dve", "act"] + (["pe"] if el % MM == 0 else [])
            best, bestv = None, None
            for ln in cands:
                r0, o0 = COST[ln]
                t = el * r0 + o0
                if ln == "pe":
                    t_new = max(load["dve"] + t, load["pe"] + el * PE_RATE)
                    m = max(t_new, load["act"])
                    t_d = t
                else:
                    m = max(load[ln] + t,
                            *(load[k] for k in load if k != ln))
                if bestv is None or m < bestv:
                    bestv, best = m, ln
            self.lane[i] = best
            r0, o0 = COST[best]
            if best == "pe":
                load["dve"] += op_elems(i) * r0 + o0
                load["pe"] = max(load["pe"], load["dve"]) + op_elems(i) * PE_RATE
            else:
                load[best] += op_elems(i) * r0 + o0
        self.load = load

        # ---- broadcast ranges (merged tails) ----
        cover = sorted(row["tail"] for row in self.rows
                       if row["tail"][1] > row["tail"][0])
        merged = []
        for s0, e0 in cover:
            if merged and s0 <= merged[-1][1]:
                merged[-1][1] = max(merged[-1][1], e0)
            else:
                merged.append([s0, e0])
        self.bcast = [(s0, e0) for s0, e0 in merged]

        self._build_core_data()
        self.pattern = (self.nrow,
                        tuple((row["suffix"], row["wh"], row["tail"])
                              for row in self.rows),
                        tuple(self.lane), tuple(self.bcast))

    def _build_core_data(self):
        nrow = self.nrow
        self.hoff = np.concatenate(
            [[0], np.cumsum([row["wh"] for row in self.rows])]).astype(int)
        HW = int(self.hoff[-1])
        self.slot_ev = np.full((NCORES, nrow, P), -1, np.int64)
        self.hg = np.full((NCORES, P, HW), NEG16, np.float16)
        self.thr = np.full((NCORES, nrow, P), BIG32, np.float32)
        self.nmid = np.full((NCORES, nrow, P), -BIG32, np.float32)
        mids = _mid32(self.t16)
        # prefix-row R sum (host side of S1)
        self.Rsum = 0
        for r, row in enumerate(self.rows):
            s, e = row["s"], row["e"]
            idx = np.arange(s, e)
            if not row["suffix"]:
                self.Rsum += int(self.R[s:e].sum())
            for k in range(NCORES):
                mine = idx[k::NCORES]
                self.slot_ev[k, r, :len(mine)] = mine
                ce = self.c[mine]
                self.thr[k, r, :len(mine)] = self.t16[mine].astype(np.float32)
                self.nmid[k, r, :len(mine)] = -mids[mine]
                h0 = self.hoff[r]
                if row["suffix"]:
                    # window [c_e, hi+1) at local x = 0..len-1
                    for p, cc in enumerate(ce):
                        L = row["hi"] + 1 - int(cc)
                        self.hg[k, p, h0:h0 + L] = self.yh16[int(cc):row["hi"] + 1]
                else:
                    # window [lo, c_e): real for x < c_e - lo
                    for p, cc in enumerate(ce):
                        L = int(cc) - row["lo"]
                        if L > 0:
                            self.hg[k, p, h0:h0 + L] = \
                                self.yh16[row["lo"]:int(cc)]

        # per-op threshold columns
        self.op_cols = []
        for i, (kind, r, x) in enumerate(self.ops):
            ln = self.lane[i]
            base = self.nmid[:, r, :] if ln == "act" else self.thr[:, r, :]
            self.op_cols.append(base.astype(np.float32))

    # ---- numpy simulation of device ops ----
    def simulate(self):
        nops = len(self.ops)
        acc = np.zeros((NCORES, P, nops), np.float64)
        for i, (kind, r, x) in enumerate(self.ops):
            ln = self.lane[i]
            h0 = self.hoff[r]
            for k in range(NCORES):
                if kind == "head":
                    data = self.hg[k][:, h0:h0 + self.rows[r]["wh"]]
                    data = data.astype(np.float32)
                else:
                    s0, e0 = x
                    data = np.broadcast_to(
                        self.yh16[s0:e0].astype(np.float32), (P, e0 - s0))
                col = self.op_cols[i][k][:, None]
                if ln == "act":
                    sg = np.where(data > -col, 1.0, -1.0)
                    acc[k, :, i] = sg.sum(axis=1)
                else:
                    acc[k, :, i] = (data >= col).sum(axis=1)
        return acc

    def combine_from_acc(self, accs):
        """accs: per-core [P, nops] raw accumulator values (simulation or
        per-partition device outputs; PE-lane ops contribute via their
        is_ge counts exactly like the dve lane)."""
        S1 = float(self.Rsum)
        for i, (kind, r, x) in enumerate(self.ops):
            L = self.rows[r]["wh"] if kind == "head" else x[1] - x[0]
            sgn = 1.0 if self.rows[r]["suffix"] else -1.0
            for k in range(NCORES):
                a = accs[k][:, i].astype(np.float64)
                if self.lane[i] == "act":
                    cntk = (L * P + a.sum()) / 2.0
                else:
                    cntk = a.sum()
                S1 += sgn * cntk
        return self._final(S1)

    def _final(self, S1):
        c_sum = np.float32(S1 - self.ns - self.OV)
        t_sum = np.float32(self.S2 - self.ns - self.OV2)
        return np.asarray(np.float32(c_sum / t_sum))

    def combine_device(self, results):
        """Device outputs: o_acc_dve/o_acc_act [P, nops] + o_ps [1, 2*MM]
        (suffix bank then prefix bank)."""
        S1 = float(self.Rsum)
        for i, (kind, r, x) in enumerate(self.ops):
            ln = self.lane[i]
            if ln == "pe":
                continue
            L = self.rows[r]["wh"] if kind == "head" else x[1] - x[0]
            sgn = 1.0 if self.rows[r]["suffix"] else -1.0
            for k in range(NCORES):
                a = results[k][f"o_acc_{ln}"][:, i].astype(np.float64)
                if ln == "act":
                    cntk = (L * P + a.sum()) / 2.0
                else:
                    cntk = a.sum()
                S1 += sgn * cntk
        for k in range(NCORES):
            ps = results[k]["o_ps"].astype(np.float64).reshape(-1)
            S1 += ps[:MM].sum() - ps[MM:].sum()
        return self._final(S1)


def build_bass(plan):
    nc = bacc.Bacc(debug=False, num_devices=NCORES)
    nrow, nops = plan.nrow, len(plan.ops)
    HW = int(plan.hoff[-1])

    ybc = nc.dram_tensor("ybc", [1, N], FP16, kind="ExternalInput")
    hg = nc.dram_tensor("hg", [P, HW], FP16, kind="ExternalInput")
    cols = nc.dram_tensor("cols", [P, nops], FP32, kind="ExternalInput")
    o_accs = {ln: nc.dram_tensor(f"o_acc_{ln}", [P, nops], FP32,
                                 kind="ExternalOutput") for ln in ("dve", "act")}
    o_ps = nc.dram_tensor("o_ps", [1, 2 * MM], FP32, kind="ExternalOutput")

    # matmul chain bookkeeping per bank (0 = suffix, 1 = prefix)
    n_mm = [0, 0]
    for i, (kind, r, x) in enumerate(plan.ops):
        if plan.lane[i] == "pe":
            L = plan.rows[r]["wh"] if kind == "head" else x[1] - x[0]
            n_mm[0 if plan.rows[r]["suffix"] else 1] += L // MM

    with tile.TileContext(nc) as tc:
        with (
            tc.tile_pool(name="c", bufs=1) as cpool,
            tc.tile_pool(name="ps", bufs=1, space="PSUM") as ppool,
        ):
            colt = cpool.tile([P, nops], FP32)
            nc.sync.dma_start(out=colt[:, :], in_=cols[:, :])
            hgt = cpool.tile([P, HW], FP16)
            # heads: split the gather DMA between two queues
            mid_r = (nrow + 1) // 2
            hsplit = int(plan.hoff[mid_r])
            if hsplit > 0:
                nc.sync.dma_start(out=hgt[:, 0:hsplit], in_=hg[:, 0:hsplit])
            bts = {}
            for (s0, e0) in plan.bcast:
                t = cpool.tile([P, e0 - s0], FP16, name=f"b{s0}")
                nc.gpsimd.dma_start(
                    out=t[:, :], in_=ybc[0:1, s0:e0].to_broadcast((P, e0 - s0)))
                bts[(s0, e0)] = t
            if hsplit < HW:
                nc.gpsimd.dma_start(out=hgt[:, hsplit:HW],
                                    in_=hg[:, hsplit:HW])

            def locate(s0, e0):
                for (bs, be), t in bts.items():
                    if s0 >= bs and e0 <= be:
                        return t[:, s0 - bs:e0 - bs]
                raise KeyError((s0, e0))

            accs = {ln: cpool.tile([P, nops], FP32, name=f"acc_{ln}")
                    for ln in ("dve", "act")}
            SW = max([TCH] + [row["wh"] for row in plan.rows])
            scratch = {ln: cpool.tile([P, SW], FP16, name=f"s_{ln}")
                       for ln in ("dve", "act")}
            zscr = [cpool.tile([P, SW], BF16, name=f"z{j}") for j in range(3)]
            ones_w = cpool.tile([P, 1], BF16)
            nc.vector.memset(ones_w[:, :], 1.0)
            banks = [ppool.tile([1, MM], FP32, name="bankS"),
                     ppool.tile([1, MM], FP32, name="bankP")]
            seen = [0, 0]
            zrot = [0]

            def emit(i, kind, r, x):
                ln = plan.lane[i]
                if kind == "head":
                    h0 = int(plan.hoff[r])
                    L = plan.rows[r]["wh"]
                    src = hgt[:, h0:h0 + L]
                else:
                    src = locate(x[0], x[1])
                    L = x[1] - x[0]
                if ln == "dve":
                    nc.vector.tensor_scalar(
                        out=scratch[ln][:, 0:L], in0=src,
                        scalar1=colt[:, i:i + 1], scalar2=0.0,
                        op0=Alu.is_ge, op1=Alu.add,
                        accum_out=accs[ln][:, i:i + 1])
                elif ln == "act":
                    nc.scalar.activation(
                        out=scratch[ln][:, 0:L], in_=src, func=ActF.Sign,
                        bias=colt[:, i:i + 1], scale=1.0,
                        accum_out=accs[ln][:, i:i + 1])
                else:  # pe lane
                    b = 0 if plan.rows[r]["suffix"] else 1
                    z = zscr[zrot[0] % 3]
                    zrot[0] += 1
                    nc.vector.tensor_scalar(
                        out=z[:, 0:L], in0=src,
                        scalar1=colt[:, i:i + 1], scalar2=None,
                        op0=Alu.is_ge)
                    for ch in range(L // MM):
                        seen[b] += 1
                        nc.tensor.matmul(
                            banks[b][0:1, 0:MM], ones_w[:, :],
                            z[:, ch * MM:(ch + 1) * MM],
                            start=(seen[b] == 1),
                            stop=(seen[b] == n_mm[b]))

            for i, (kind, r, x) in enumerate(plan.ops):
                emit(i, kind, r, x)

            stg = cpool.tile([1, 2 * MM], FP32)
            for b in range(2):
                if n_mm[b] == 0:
                    nc.vector.memset(stg[:, b * MM:(b + 1) * MM], 0.0)
                else:
                    nc.vector.tensor_copy(out=stg[:, b * MM:(b + 1) * MM],
                                          in_=banks[b][0:1, :])
            nc.sync.dma_start(out=o_ps[:, :], in_=stg[:, :])
            for ln in ("dve", "act"):
                nc.sync.dma_start(out=o_accs[ln][:, :], in_=accs[ln][:, :])

    nc.compile()
    return nc


_NC_CACHE = {}


def _get_nc(plan):
    key = plan.pattern
    if key not in _NC_CACHE:
        _NC_CACHE[key] = build_bass(plan)
    return _NC_CACHE[key]


def kernel(y, y_hat, status, _run_kwargs=None, _simulate=False):
    plan = Plan(y, y_hat, status)
    if _simulate:
        acc = plan.simulate()
        return plan.combine_from_acc([acc[k] for k in range(NCORES)])
    nc = _get_nc(plan)
    ybc2 = np.ascontiguousarray(plan.yh16.reshape(1, N))
    in_maps = []
    for k in range(NCORES):
        in_maps.append({
            "ybc": ybc2,
            "hg": np.ascontiguousarray(plan.hg[k]),
            "cols": np.ascontiguousarray(
                np.stack([plan.op_cols[i][k] for i in range(len(plan.ops))],
                         axis=1)),
        })
    kw = dict(_run_kwargs or {})
    res = bass_utils.run_bass_kernel_spmd(
        nc, in_maps, core_ids=list(range(NCORES)), **kw)
    out = plan.combine_device(res.results)
    if _run_kwargs is not None:
        return out, res
    return out


if __name__ == "__main__":
    rng = np.random.default_rng(0)
    y = rng.standard_normal(N).astype(np.float32)
    yh = rng.standard_normal(N).astype(np.float32)
    st = (rng.integers(0, 2, N)).astype(np.int32)
    print(kernel(y, yh, st, _simulate=True))


# revision 17
# speedup vs baseline: 1.7553x; 1.0575x over previous
"""Concordance-index (C-index) kernel for Trainium2, 8 NeuronCores — v4.2.

Math
----
Reference computes, over all pairs i<j of N=16384 samples:
    cc = ((y_i>=y_j & yh_i>=yh_j & st_j) | (y_i<=y_j & yh_i<=yh_j & st_i)) & triu
    tp = ((y_i<=y_j & st_i) | (y_i>=y_j & st_j)) & triu
    out = sum(cc) / sum(tp)

With A_ae = [y_a >= y_e], B_ae = [yh_a >= yh_e] over (a = all, e = event):
    sum(cc) = S1 - ns - OV,   S1 = sum_e T_e,  T_e = sum_a A_ae * B_ae
    sum(tp) = S2 - ns - OV2,  S2 = sum_e sum_a A_ae
where OV / OV2 are exact-fp32-tie corrections among event pairs.

Key ideas
---------
* Sort the a-axis by fp32 y: A_ae = [a >= c_e], c_e host-exact, so S2 is
  pure host math and T_e is a 1-D count of [yh_a >= t_e] over the suffix
  [c_e, N) — or R_e (host-known total) minus the prefix count.  Each
  event pays only its SHORTER side: ~N^2/8 compares total.
* Events sorted by c, 1024 per row (128/core x 8 cores).  The ragged
  per-event boundary is absorbed into a HOST-GATHERED window per event
  (padded with -60000 -> compares contribute 0); the rest is a
  row-uniform 64-aligned tail.  Every device op is a plain fused
  compare+accumulate — no masks, no products.
* S1 only needs GLOBAL sums, so three lanes run concurrently:
  ACT (Sign+accum), DVE (is_ge+add fused), and DVE plain is_ge in 4x
  DVE perf mode feeding PE ones-matmul column reductions in PSUM.
* DMA is chunked over three queue rings (sync/gpsimd/tensor) in the
  order ops consume it; ops are emitted in estimated-arrival order.
* fp16 compare semantics host+device; only fp16-vs-fp32 tie noise
  remains (~1e-4 rel; gate 2e-2).
"""

import math
import os
import sys

import numpy as np

for _p in ("/opt/trn_rl_repo", "/root/.axon_site", "/root/.axon_site/_ro/trn_rl_repo"):
    if os.path.isdir(_p) and _p not in sys.path:
        sys.path.append(_p)

import ml_dtypes  # noqa: F401  (env check)

import concourse.bacc as bacc
import concourse.mybir as mybir
from concourse import bass_utils
from concourse import tile

N = 16384
P = 128
NCORES = 8
NEG16 = np.float16(-60000.0)
BIG32 = np.float32(1e30)
TCH = 3072          # tail/bcast chunk width
MM = 512            # PE matmul moving width (one PSUM bank)

FP32 = mybir.dt.float32
FP16 = mybir.dt.float16
BF16 = mybir.dt.bfloat16
Alu = mybir.AluOpType
ActF = mybir.ActivationFunctionType

# measured ns/elem and ns/op overhead (TRN2, aligned [128, >=2048] ops)
COST = {
    "dve": (1.09, 220.0),    # tensor_scalar is_ge+add fused accum (1x)
    "act": (0.95, 300.0),    # activation Sign with accum
    "pe": (0.29, 130.0),     # DVE plain is_ge at 4x (feeds PE)
}
PE_RATE = 1.05               # ns/elem ones-matmul reduce incl ldweights
DMA_NSPB = 2.9e-3            # ns per byte per ring (~345 GB/s)


def _mid32(t16):
    prev = np.nextafter(t16, np.float16(-np.inf))
    return (t16.astype(np.float32) + prev.astype(np.float32)) * np.float32(0.5)


class Plan:
    def __init__(self, y, yh, status):
        y32 = np.asarray(y, np.float32)
        yh32 = np.asarray(yh, np.float32)
        st = np.asarray(status)

        order = np.argsort(y32, kind="stable")
        ys = y32[order]
        self.yh16 = yh32[order].astype(np.float16)

        ev = np.nonzero(st == 1)[0]
        self.ns = int(len(ev))
        t16 = yh32[ev].astype(np.float16)
        c = np.searchsorted(ys, y32[ev], side="left").astype(np.int64)
        yh16s = np.sort(self.yh16)
        R = (N - np.searchsorted(yh16s, t16, side="left")).astype(np.int64)
        self.S2 = int((np.int64(N) - c).sum())

        yv = y32[ev]
        _, cnt = np.unique(yv, return_counts=True)
        self.OV2 = int((cnt.astype(np.int64) * (cnt - 1) // 2).sum())
        pair = np.stack([yv, yh32[ev]], 1)
        _, cnt2 = np.unique(pair, axis=0, return_counts=True)
        self.OV = int((cnt2.astype(np.int64) * (cnt2 - 1) // 2).sum())

        eo = np.argsort(c, kind="stable")
        self.c = c[eo]
        self.t16 = t16[eo]
        self.R = R[eo]

        self.nrow = max(1, math.ceil(self.ns / (P * NCORES)))
        self.rows = []
        for r in range(self.nrow):
            s, e = r * P * NCORES, min((r + 1) * P * NCORES, self.ns)
            cr = self.c[s:e]
            lo, hi = int(cr.min()), int(cr.max())
            suffix = int((np.int64(N) - cr).sum()) <= int(cr.sum())
            if suffix:
                a = min(N, ((hi + 1 + 63) // 64) * 64)  # 64-aligned tail start
                wh = a - lo                   # head window [c_e, a)
                tail = (a, N)
            else:
                b = (lo // 64) * 64           # 64-aligned tail end
                wh = hi - b                   # head window [b, c_e)
                tail = (0, b)
            wh = max(wh, 1)
            wh = ((wh + MM - 1) // MM) * MM   # 512-pad for the PE lane
            self.rows.append({"suffix": suffix, "s": s, "e": e, "lo": lo,
                              "hi": hi, "wh": wh, "tail": tail,
                              "abound": a if suffix else b})

        # ---- broadcast ranges (merged tails), split into chunk tiles ----
        cover = sorted(row["tail"] for row in self.rows
                       if row["tail"][1] > row["tail"][0])
        merged = []
        for s0, e0 in cover:
            if merged and s0 <= merged[-1][1]:
                merged[-1][1] = max(merged[-1][1], e0)
            else:
                merged.append([s0, e0])
        self.chunks = []        # (start, end) bcast tiles
        for s0, e0 in merged:
            x = s0
            while x < e0:
                self.chunks.append((x, min(x + TCH, e0)))
                x += TCH

        # ---- op list: heads + tail pieces cut at chunk boundaries ----
        self.ops = []           # (kind, row, (start, end) or None)
        for r, row in enumerate(self.rows):
            self.ops.append(("head", r, None))
            t0, t1 = row["tail"]
            for (cs, ce) in self.chunks:
                s0, e0 = max(t0, cs), min(t1, ce)
                if s0 < e0:
                    self.ops.append(("tail", r, (s0, e0)))

        def op_elems(i):
            kind, r, x = self.ops[i]
            return self.rows[r]["wh"] if kind == "head" else x[1] - x[0]

        self.op_elems = op_elems

        # ---- DMA schedule: interleave head segments and bcast chunks
        # round-robin over 3 rings (sync / gpsimd / tensor) ----
        self.hoff = np.concatenate(
            [[0], np.cumsum([row["wh"] for row in self.rows])]).astype(int)
        HW = int(self.hoff[-1])
        self.HW = HW
        dma_items = [("cols", None)]
        hi_, ci_ = 0, 0
        while hi_ < self.nrow or ci_ < len(self.chunks):
            if hi_ < self.nrow:
                dma_items.append(("head", hi_))
                hi_ += 1
            if ci_ < len(self.chunks):
                dma_items.append(("chunk", self.chunks[ci_]))
                ci_ += 1
        ring_t = [0.0, 0.0, 0.0]
        self.dma_ring = {}
        self.head_arr = [0.0] * self.nrow
        self.chunk_arr = {}
        for j, (knd, val) in enumerate(dma_items):
            # ring 2 is the ACT queue: only early items (issue cost there
            # is hidden behind the initial data wait)
            rg = j % 3 if j < 9 else j % 2
            if knd == "cols":
                sz = len(self.ops) * P * 4
            elif knd == "head":
                sz = int(self.rows[val]["wh"]) * P * 2
            else:
                sz = (val[1] - val[0]) * P * 2
            ring_t[rg] += sz * DMA_NSPB
            self.dma_ring[(knd, val)] = rg
            if knd == "head":
                self.head_arr[val] = ring_t[rg]
            elif knd == "chunk":
                self.chunk_arr[val] = ring_t[rg]
        self.dma_items = dma_items

        def op_arrival(i):
            kind, r, x = self.ops[i]
            if kind == "head":
                return self.head_arr[r]
            for (cs, ce) in self.chunks:
                if x[0] >= cs and x[1] <= ce:
                    return self.chunk_arr[(cs, ce)]
            raise KeyError(x)

        # ---- lane assignment: quota split (LP balance), arrival order ----
        E = float(sum(op_elems(i) for i in range(len(self.ops))))
        rd, ra, rz = COST["dve"][0], COST["act"][0], COST["pe"][0]
        # act*x = dve: rd*y + rz*z = pe: PE_RATE*z = T ; x+y+z = E
        T = E / (1.0 / ra + (1.0 - rz / PE_RATE) / rd + 1.0 / PE_RATE)
        quota = {"act": T / ra, "pe": T / PE_RATE}
        quota["dve"] = E - quota["act"] - quota["pe"]
        order_i = sorted(range(len(self.ops)), key=op_arrival)
        used = {"dve": 0.0, "act": 0.0, "pe": 0.0}
        load = {"dve": 0.0, "act": 0.0, "pe": 0.0}
        self.lane = [None] * len(self.ops)
        for i in order_i:
            el = op_elems(i)
            arr = op_arrival(i)
            cands = ["dve", "act"] + (["pe"] if el % MM == 0 else [])
            best = max(cands, key=lambda ln: quota[ln] - used[ln])
            self.lane[i] = best
            used[best] += el
            r0, o0 = COST[best]
            if best == "pe":
                t0 = max(load["dve"], arr) + el * r0 + o0
                load["dve"] = t0
                load["pe"] = max(load["pe"], t0) + el * PE_RATE
            else:
                load[best] = max(load[best], arr) + el * r0 + o0
        self.load = load
        self.emit_order = order_i

        self._build_core_data()
        self.pattern = (self.nrow,
                        tuple((row["suffix"], row["wh"], row["tail"])
                              for row in self.rows),
                        tuple(self.lane), tuple(self.chunks))

    def _build_core_data(self):
        nrow = self.nrow
        HW = self.HW
        self.slot_ev = np.full((NCORES, nrow, P), -1, np.int64)
        self.hg = np.full((NCORES, P, HW), NEG16, np.float16)
        self.thr = np.full((NCORES, nrow, P), BIG32, np.float32)
        self.nmid = np.full((NCORES, nrow, P), -BIG32, np.float32)
        mids = _mid32(self.t16)
        self.Rsum = 0
        for r, row in enumerate(self.rows):
            s, e = row["s"], row["e"]
            idx = np.arange(s, e)
            if not row["suffix"]:
                self.Rsum += int(self.R[s:e].sum())
            ab = row["abound"]
            for k in range(NCORES):
                mine = idx[k::NCORES]
                self.slot_ev[k, r, :len(mine)] = mine
                ce = self.c[mine]
                self.thr[k, r, :len(mine)] = self.t16[mine].astype(np.float32)
                self.nmid[k, r, :len(mine)] = -mids[mine]
                h0 = int(self.hoff[r])
                if row["suffix"]:
                    for p, cc in enumerate(ce):
                        L = ab - int(cc)          # head [c_e, a)
                        if L > 0:
                            self.hg[k, p, h0:h0 + L] = \
                                self.yh16[int(cc):ab]
                else:
                    for p, cc in enumerate(ce):
                        L = int(cc) - ab          # head [b, c_e)
                        if L > 0:
                            self.hg[k, p, h0:h0 + L] = \
                                self.yh16[ab:int(cc)]

        self.op_cols = []
        for i, (kind, r, x) in enumerate(self.ops):
            ln = self.lane[i]
            base = self.nmid[:, r, :] if ln == "act" else self.thr[:, r, :]
            self.op_cols.append(base.astype(np.float32))

    # ---- numpy simulation ----
    def simulate(self):
        nops = len(self.ops)
        acc = np.zeros((NCORES, P, nops), np.float64)
        for i, (kind, r, x) in enumerate(self.ops):
            ln = self.lane[i]
            h0 = int(self.hoff[r])
            for k in range(NCORES):
                if kind == "head":
                    data = self.hg[k][:, h0:h0 + self.rows[r]["wh"]]
                    data = data.astype(np.float32)
                else:
                    s0, e0 = x
                    data = np.broadcast_to(
                        self.yh16[s0:e0].astype(np.float32), (P, e0 - s0))
                col = self.op_cols[i][k][:, None]
                if ln == "act":
                    sg = np.where(data > -col, 1.0, -1.0)
                    acc[k, :, i] = sg.sum(axis=1)
                else:
                    acc[k, :, i] = (data >= col).sum(axis=1)
        return acc

    def combine_from_acc(self, accs):
        S1 = float(self.Rsum)
        for i, (kind, r, x) in enumerate(self.ops):
            L = self.rows[r]["wh"] if kind == "head" else x[1] - x[0]
            sgn = 1.0 if self.rows[r]["suffix"] else -1.0
            for k in range(NCORES):
                a = accs[k][:, i].astype(np.float64)
                if self.lane[i] == "act":
                    cntk = (L * P + a.sum()) / 2.0
                else:
                    cntk = a.sum()
                S1 += sgn * cntk
        return self._final(S1)

    def _final(self, S1):
        c_sum = np.float32(S1 - self.ns - self.OV)
        t_sum = np.float32(self.S2 - self.ns - self.OV2)
        return np.asarray(np.float32(c_sum / t_sum))

    def combine_device(self, results):
        S1 = float(self.Rsum)
        for i, (kind, r, x) in enumerate(self.ops):
            ln = self.lane[i]
            if ln == "pe":
                continue
            L = self.rows[r]["wh"] if kind == "head" else x[1] - x[0]
            sgn = 1.0 if self.rows[r]["suffix"] else -1.0
            for k in range(NCORES):
                a = results[k][f"o_acc_{ln}"][:, i].astype(np.float64)
                if ln == "act":
                    cntk = (L * P + a.sum()) / 2.0
                else:
                    cntk = a.sum()
                S1 += sgn * cntk
        for k in range(NCORES):
            ps = results[k]["o_ps"].astype(np.float64).reshape(-1)
            S1 += ps[:MM].sum() - ps[MM:].sum()
        return self._final(S1)


def build_bass(plan):
    nc = bacc.Bacc(debug=False, num_devices=NCORES)
    nrow, nops = plan.nrow, len(plan.ops)
    HW = plan.HW

    ybc = nc.dram_tensor("ybc", [1, N], FP16, kind="ExternalInput")
    hg = nc.dram_tensor("hg", [P, HW], FP16, kind="ExternalInput")
    cols = nc.dram_tensor("cols", [P, nops], FP32, kind="ExternalInput")
    o_accs = {ln: nc.dram_tensor(f"o_acc_{ln}", [P, nops], FP32,
                                 kind="ExternalOutput")
              for ln in ("dve", "act")}
    o_ps = nc.dram_tensor("o_ps", [1, 2 * MM], FP32, kind="ExternalOutput")

    n_mm = [0, 0]
    for i, (kind, r, x) in enumerate(plan.ops):
        if plan.lane[i] == "pe":
            L = plan.rows[r]["wh"] if kind == "head" else x[1] - x[0]
            n_mm[0 if plan.rows[r]["suffix"] else 1] += L // MM

    rings = {0: nc.sync, 1: nc.gpsimd, 2: nc.scalar}

    with tile.TileContext(nc) as tc:
        with (
            tc.tile_pool(name="c", bufs=1) as cpool,
            tc.tile_pool(name="ps", bufs=1, space="PSUM") as ppool,
        ):
            colt = cpool.tile([P, nops], FP32)
            hgt = cpool.tile([P, HW], FP16)
            bts = {}
            for (knd, val) in plan.dma_items:
                rg = rings[plan.dma_ring[(knd, val)]]
                if knd == "cols":
                    rg.dma_start(out=colt[:, :], in_=cols[:, :])
                elif knd == "head":
                    h0, h1 = int(plan.hoff[val]), int(plan.hoff[val + 1])
                    rg.dma_start(out=hgt[:, h0:h1], in_=hg[:, h0:h1])
                else:
                    cs, ce = val
                    t = cpool.tile([P, ce - cs], FP16, name=f"b{cs}")
                    rg.dma_start(
                        out=t[:, :],
                        in_=ybc[0:1, cs:ce].to_broadcast((P, ce - cs)))
                    bts[(cs, ce)] = t

            def locate(s0, e0):
                for (bs, be), t in bts.items():
                    if s0 >= bs and e0 <= be:
                        return t[:, s0 - bs:e0 - bs]
                raise KeyError((s0, e0))

            accs = {ln: cpool.tile([P, nops], FP32, name=f"acc_{ln}")
                    for ln in ("dve", "act")}
            SW = max([TCH] + [row["wh"] for row in plan.rows])
            scratch = {ln: cpool.tile([P, SW], FP16, name=f"s_{ln}")
                       for ln in ("dve", "act")}
            zscr = [cpool.tile([P, SW], BF16, name=f"z{j}") for j in range(3)]
            ones_w = cpool.tile([P, 1], BF16)
            nc.vector.memset(ones_w[:, :], 1.0)
            banks = [ppool.tile([1, MM], FP32, name="bankS"),
                     ppool.tile([1, MM], FP32, name="bankP")]
            seen = [0, 0]
            zrot = [0]

            def emit(i):
                kind, r, x = plan.ops[i]
                ln = plan.lane[i]
                if kind == "head":
                    h0 = int(plan.hoff[r])
                    L = plan.rows[r]["wh"]
                    src = hgt[:, h0:h0 + L]
                else:
                    src = locate(x[0], x[1])
                    L = x[1] - x[0]
                if ln == "dve":
                    nc.vector.tensor_scalar(
                        out=scratch[ln][:, 0:L], in0=src,
                        scalar1=colt[:, i:i + 1], scalar2=0.0,
                        op0=Alu.is_ge, op1=Alu.add,
                        accum_out=accs[ln][:, i:i + 1])
                elif ln == "act":
                    nc.scalar.activation(
                        out=scratch[ln][:, 0:L], in_=src, func=ActF.Sign,
                        bias=colt[:, i:i + 1], scale=1.0,
                        accum_out=accs[ln][:, i:i + 1])
                else:
                    b = 0 if plan.rows[r]["suffix"] else 1
                    z = zscr[zrot[0] % 3]
                    zrot[0] += 1
                    nc.vector.tensor_scalar(
                        out=z[:, 0:L], in0=src,
                        scalar1=colt[:, i:i + 1], scalar2=None,
                        op0=Alu.is_ge)
                    for ch in range(L // MM):
                        seen[b] += 1
                        nc.tensor.matmul(
                            banks[b][0:1, 0:MM], ones_w[:, :],
                            z[:, ch * MM:(ch + 1) * MM],
                            start=(seen[b] == 1),
                            stop=(seen[b] == n_mm[b]))

            for i in plan.emit_order:
                emit(i)

            stg = cpool.tile([1, 2 * MM], FP32)
            for b in range(2):
                if n_mm[b] == 0:
                    nc.vector.memset(stg[:, b * MM:(b + 1) * MM], 0.0)
                else:
                    nc.vector.tensor_copy(out=stg[:, b * MM:(b + 1) * MM],
                                          in_=banks[b][0:1, :])
            rings[0].dma_start(out=o_ps[:, :], in_=stg[:, :])
            for ln in ("dve", "act"):
                rings[0].dma_start(out=o_accs[ln][:, :], in_=accs[ln][:, :])

    nc.compile()
    return nc


_NC_CACHE = {}


def _get_nc(plan):
    key = plan.pattern
    if key not in _NC_CACHE:
        _NC_CACHE[key] = build_bass(plan)
    return _NC_CACHE[key]


def kernel(y, y_hat, status, _run_kwargs=None, _simulate=False):
    plan = Plan(y, y_hat, status)
    if _simulate:
        acc = plan.simulate()
        return plan.combine_from_acc([acc[k] for k in range(NCORES)])
    nc = _get_nc(plan)
    ybc2 = np.ascontiguousarray(plan.yh16.reshape(1, N))
    in_maps = []
    for k in range(NCORES):
        in_maps.append({
            "ybc": ybc2,
            "hg": np.ascontiguousarray(plan.hg[k]),
            "cols": np.ascontiguousarray(
                np.stack([plan.op_cols[i][k] for i in range(len(plan.ops))],
                         axis=1)),
        })
    kw = dict(_run_kwargs or {})
    res = bass_utils.run_bass_kernel_spmd(
        nc, in_maps, core_ids=list(range(NCORES)), **kw)
    out = plan.combine_device(res.results)
    if _run_kwargs is not None:
        return out, res
    return out


if __name__ == "__main__":
    rng = np.random.default_rng(0)
    y = rng.standard_normal(N).astype(np.float32)
    yh = rng.standard_normal(N).astype(np.float32)
    st = (rng.integers(0, 2, N)).astype(np.int32)
    print(kernel(y, yh, st, _simulate=True))


# revision 20
# speedup vs baseline: 1.8765x; 1.0691x over previous
"""Concordance-index (C-index) kernel for Trainium2, 8 NeuronCores — v4.2.

Math
----
Reference computes, over all pairs i<j of N=16384 samples:
    cc = ((y_i>=y_j & yh_i>=yh_j & st_j) | (y_i<=y_j & yh_i<=yh_j & st_i)) & triu
    tp = ((y_i<=y_j & st_i) | (y_i>=y_j & st_j)) & triu
    out = sum(cc) / sum(tp)

With A_ae = [y_a >= y_e], B_ae = [yh_a >= yh_e] over (a = all, e = event):
    sum(cc) = S1 - ns - OV,   S1 = sum_e T_e,  T_e = sum_a A_ae * B_ae
    sum(tp) = S2 - ns - OV2,  S2 = sum_e sum_a A_ae
where OV / OV2 are exact-fp32-tie corrections among event pairs.

Key ideas
---------
* Sort the a-axis by fp32 y: A_ae = [a >= c_e], c_e host-exact, so S2 is
  pure host math and T_e is a 1-D count of [yh_a >= t_e] over the suffix
  [c_e, N) — or R_e (host-known total) minus the prefix count.  Each
  event pays only its SHORTER side: ~N^2/8 compares total.
* Events sorted by c, 1024 per row (128/core x 8 cores).  The ragged
  per-event boundary is absorbed into a HOST-GATHERED window per event
  (padded with -60000 -> compares contribute 0); the rest is a
  row-uniform 64-aligned tail.  Every device op is a plain fused
  compare+accumulate — no masks, no products.
* S1 only needs GLOBAL sums, so three lanes run concurrently:
  ACT (Sign+accum), DVE (is_ge+add fused), and DVE plain is_ge in 4x
  DVE perf mode feeding PE ones-matmul column reductions in PSUM.
* DMA is chunked over three queue rings (sync/gpsimd/tensor) in the
  order ops consume it; ops are emitted in estimated-arrival order.
* fp16 compare semantics host+device; only fp16-vs-fp32 tie noise
  remains (~1e-4 rel; gate 2e-2).
"""

import math
import os
import sys

import numpy as np

for _p in ("/opt/trn_rl_repo", "/root/.axon_site", "/root/.axon_site/_ro/trn_rl_repo"):
    if os.path.isdir(_p) and _p not in sys.path:
        sys.path.append(_p)

import ml_dtypes  # noqa: F401  (env check)

import concourse.bacc as bacc
import concourse.mybir as mybir
from concourse import bass_utils
from concourse import tile

N = 16384
P = 128
NCORES = 8
NEG16 = np.float16(-60000.0)
BIG32 = np.float32(1e30)
TCH = 3072          # tail/bcast chunk width
MM = 512            # PE matmul moving width (one PSUM bank)

FP32 = mybir.dt.float32
FP16 = mybir.dt.float16
BF16 = mybir.dt.bfloat16
Alu = mybir.AluOpType
ActF = mybir.ActivationFunctionType

# measured ns/elem and ns/op overhead (TRN2, aligned [128, >=2048] ops)
COST = {
    "dve": (1.09, 220.0),    # tensor_scalar is_ge+add fused accum (1x)
    "act": (0.95, 300.0),    # activation Sign with accum
    "pe": (0.29, 130.0),     # DVE plain is_ge at 4x (feeds PE)
}
PE_RATE = 1.05               # ns/elem ones-matmul reduce incl ldweights
DMA_NAT = 6.1e-3             # ns/SBUF-byte, natural DMA (~163 GB/s, HBM-read bound)
DMA_BC = 1.9e-3              # ns/SBUF-byte, broadcast DMA (~530 GB/s)


def _mid32(t16):
    prev = np.nextafter(t16, np.float16(-np.inf))
    return (t16.astype(np.float32) + prev.astype(np.float32)) * np.float32(0.5)


class Plan:
    def __init__(self, y, yh, status):
        y32 = np.asarray(y, np.float32)
        yh32 = np.asarray(yh, np.float32)
        st = np.asarray(status)

        order = np.argsort(y32, kind="stable")
        ys = y32[order]
        self.yh16 = yh32[order].astype(np.float16)

        ev = np.nonzero(st == 1)[0]
        self.ns = int(len(ev))
        t16 = yh32[ev].astype(np.float16)
        c = np.searchsorted(ys, y32[ev], side="left").astype(np.int64)
        yh16s = np.sort(self.yh16)
        R = (N - np.searchsorted(yh16s, t16, side="left")).astype(np.int64)
        self.S2 = int((np.int64(N) - c).sum())

        yv = y32[ev]
        _, cnt = np.unique(yv, return_counts=True)
        self.OV2 = int((cnt.astype(np.int64) * (cnt - 1) // 2).sum())
        pair = np.stack([yv, yh32[ev]], 1)
        _, cnt2 = np.unique(pair, axis=0, return_counts=True)
        self.OV = int((cnt2.astype(np.int64) * (cnt2 - 1) // 2).sum())

        eo = np.argsort(c, kind="stable")
        self.c = c[eo]
        self.t16 = t16[eo]
        self.R = R[eo]

        self.nrow = max(1, math.ceil(self.ns / (P * NCORES)))
        self.rows = []
        for r in range(self.nrow):
            s, e = r * P * NCORES, min((r + 1) * P * NCORES, self.ns)
            cr = self.c[s:e]
            lo, hi = int(cr.min()), int(cr.max())
            suffix = int((np.int64(N) - cr).sum()) <= int(cr.sum())
            if suffix:
                a = min(N, ((hi + 1 + MM - 1) // MM) * MM)  # 512-aligned
                tail = (a, N)
                sel = [j for j in range((lo + MM - 1) // MM, a // MM)]
            else:
                b = (lo // MM) * MM
                tail = (0, b)
                sel = [j for j in range(b // MM, N // MM)
                       if MM * (j + 1) <= hi]
            self.rows.append({"suffix": suffix, "s": s, "e": e, "lo": lo,
                              "hi": hi, "wh": MM, "tail": tail, "sel": sel,
                              "abound": a if suffix else b})

        # ---- broadcast ranges: tails + selective blocks, merged ----
        cover = [row["tail"] for row in self.rows
                 if row["tail"][1] > row["tail"][0]]
        for row in self.rows:
            for j in row["sel"]:
                cover.append((j * MM, (j + 1) * MM))
        cover.sort()
        merged = []
        for s0, e0 in cover:
            if merged and s0 <= merged[-1][1]:
                merged[-1][1] = max(merged[-1][1], e0)
            else:
                merged.append([s0, e0])
        self.chunks = []        # (start, end) bcast tiles
        for s0, e0 in merged:
            x = s0
            while x < e0:
                self.chunks.append((x, min(x + TCH, e0)))
                x += TCH

        # ---- op list: partial-head + selective blocks + tail pieces ----
        self.ops = []           # (kind, row, (start, end) or None)
        for r, row in enumerate(self.rows):
            self.ops.append(("head", r, None))
            for j in row["sel"]:
                self.ops.append(("sel", r, (j * MM, (j + 1) * MM)))
            t0, t1 = row["tail"]
            for (cs, ce) in self.chunks:
                s0, e0 = max(t0, cs), min(t1, ce)
                if s0 < e0:
                    self.ops.append(("tail", r, (s0, e0)))

        def op_elems(i):
            kind, r, x = self.ops[i]
            return self.rows[r]["wh"] if kind == "head" else x[1] - x[0]

        self.op_elems = op_elems

        # ---- DMA schedule: interleave head segments and bcast chunks
        # round-robin over 3 rings (sync / gpsimd / tensor) ----
        self.hoff = np.concatenate(
            [[0], np.cumsum([row["wh"] for row in self.rows])]).astype(int)
        HW = int(self.hoff[-1])
        self.HW = HW
        dma_items = [("cols", None)]
        hi_, ci_ = 0, 0
        while hi_ < self.nrow or ci_ < len(self.chunks):
            if hi_ < self.nrow:
                dma_items.append(("head", hi_))
                hi_ += 1
            if ci_ < len(self.chunks):
                dma_items.append(("chunk", self.chunks[ci_]))
                ci_ += 1
        ring_t = [0.0, 0.0, 0.0]
        self.dma_ring = {}
        self.head_arr = [0.0] * self.nrow
        self.chunk_arr = {}
        for j, (knd, val) in enumerate(dma_items):
            # ring 2 is the ACT queue: only early items (issue cost there
            # is hidden behind the initial data wait)
            rg = j % 3 if j < 9 else j % 2
            if knd == "cols":
                sz = len(self.ops) * P * 4
                rate = DMA_NAT
            elif knd == "head":
                sz = int(self.rows[val]["wh"]) * P * 2
                rate = DMA_NAT
            else:
                sz = (val[1] - val[0]) * P * 2
                rate = DMA_BC
            ring_t[rg] += sz * rate
            self.dma_ring[(knd, val)] = rg
            if knd == "head":
                self.head_arr[val] = ring_t[rg]
            elif knd == "chunk":
                self.chunk_arr[val] = ring_t[rg]
        self.dma_items = dma_items

        def op_arrival(i):
            kind, r, x = self.ops[i]
            if kind == "head":
                return self.head_arr[r]
            for (cs, ce) in self.chunks:
                if x[0] >= cs and x[1] <= ce:
                    return self.chunk_arr[(cs, ce)]
            raise KeyError((kind, r, x))

        # ---- lane assignment: quota split (LP balance), arrival order ----
        E = float(sum(op_elems(i) for i in range(len(self.ops))))
        rd, ra, rz = COST["dve"][0], COST["act"][0], COST["pe"][0]
        # act*x = dve: rd*y + rz*z = pe: PE_RATE*z = T ; x+y+z = E
        T = E / (1.0 / ra + (1.0 - rz / PE_RATE) / rd + 1.0 / PE_RATE)
        quota = {"act": T / ra, "pe": T / PE_RATE}
        quota["dve"] = E - quota["act"] - quota["pe"]
        order_i = sorted(range(len(self.ops)), key=op_arrival)
        used = {"dve": 0.0, "act": 0.0, "pe": 0.0}
        load = {"dve": 0.0, "act": 0.0, "pe": 0.0}
        self.lane = [None] * len(self.ops)
        for i in order_i:
            el = op_elems(i)
            arr = op_arrival(i)
            cands = ["dve", "act"] + (["pe"] if el % MM == 0 else [])
            best = max(cands, key=lambda ln: quota[ln] - used[ln])
            self.lane[i] = best
            used[best] += el
            r0, o0 = COST[best]
            if best == "pe":
                t0 = max(load["dve"], arr) + el * r0 + o0
                load["dve"] = t0
                load["pe"] = max(load["pe"], t0) + el * PE_RATE
            else:
                load[best] = max(load[best], arr) + el * r0 + o0
        self.load = load
        self.emit_order = order_i

        self._build_core_data()
        self.pattern = (self.nrow,
                        tuple((row["suffix"], row["wh"], row["tail"],
                               tuple(row["sel"])) for row in self.rows),
                        tuple(self.lane), tuple(self.chunks))

    def _build_core_data(self):
        nrow = self.nrow
        HW = self.HW
        self.slot_ev = np.full((NCORES, nrow, P), -1, np.int64)
        self.cslot = np.full((NCORES, nrow, P), -1, np.int64)
        self.hg = np.full((NCORES, P, HW), NEG16, np.float16)
        self.thr = np.full((NCORES, nrow, P), BIG32, np.float32)
        self.nmid = np.full((NCORES, nrow, P), -BIG32, np.float32)
        mids = _mid32(self.t16)
        self.Rsum = 0
        for r, row in enumerate(self.rows):
            s, e = row["s"], row["e"]
            idx = np.arange(s, e)
            if not row["suffix"]:
                self.Rsum += int(self.R[s:e].sum())
            for k in range(NCORES):
                mine = idx[k::NCORES]
                self.slot_ev[k, r, :len(mine)] = mine
                ce = self.c[mine]
                self.cslot[k, r, :len(mine)] = ce
                self.thr[k, r, :len(mine)] = self.t16[mine].astype(np.float32)
                self.nmid[k, r, :len(mine)] = -mids[mine]
                h0 = int(self.hoff[r])
                if row["suffix"]:
                    # partial head [c_e, MM*ceil(c_e/MM))
                    for p, cc in enumerate(ce):
                        a = int(-(-int(cc) // MM)) * MM
                        L = min(a, N) - int(cc)
                        if L > 0:
                            self.hg[k, p, h0:h0 + L] = \
                                self.yh16[int(cc):int(cc) + L]
                else:
                    # partial head [MM*floor(c_e/MM), c_e)
                    for p, cc in enumerate(ce):
                        b = (int(cc) // MM) * MM
                        L = int(cc) - b
                        if L > 0:
                            self.hg[k, p, h0:h0 + L] = \
                                self.yh16[b:int(cc)]

        self.op_cols = []
        for i, (kind, r, x) in enumerate(self.ops):
            ln = self.lane[i]
            base = self.nmid[:, r, :] if ln == "act" else self.thr[:, r, :]
            if kind == "sel":
                j = x[0] // MM
                cs = self.cslot[:, r, :]
                if self.rows[r]["suffix"]:
                    active = (cs >= 0) & (cs <= j * MM)
                else:
                    active = (cs >= 0) & (cs >= (j + 1) * MM)
                maskv = np.float32(-BIG32) if ln == "act" else BIG32
                col = np.where(active, base, maskv).astype(np.float32)
            else:
                col = base.astype(np.float32)
            self.op_cols.append(col)

    # ---- numpy simulation ----
    def simulate(self):
        nops = len(self.ops)
        acc = np.zeros((NCORES, P, nops), np.float64)
        for i, (kind, r, x) in enumerate(self.ops):
            ln = self.lane[i]
            h0 = int(self.hoff[r])
            for k in range(NCORES):
                if kind == "head":
                    data = self.hg[k][:, h0:h0 + self.rows[r]["wh"]]
                    data = data.astype(np.float32)
                else:
                    s0, e0 = x
                    data = np.broadcast_to(
                        self.yh16[s0:e0].astype(np.float32), (P, e0 - s0))
                col = self.op_cols[i][k][:, None]
                if ln == "act":
                    sg = np.where(data > -col, 1.0, -1.0)
                    acc[k, :, i] = sg.sum(axis=1)
                else:
                    acc[k, :, i] = (data >= col).sum(axis=1)
        return acc

    def combine_from_acc(self, accs):
        S1 = float(self.Rsum)
        for i, (kind, r, x) in enumerate(self.ops):
            L = self.rows[r]["wh"] if kind == "head" else x[1] - x[0]
            sgn = 1.0 if self.rows[r]["suffix"] else -1.0
            for k in range(NCORES):
                a = accs[k][:, i].astype(np.float64)
                if self.lane[i] == "act":
                    cntk = (L * P + a.sum()) / 2.0
                else:
                    cntk = a.sum()
                S1 += sgn * cntk
        return self._final(S1)

    def _final(self, S1):
        c_sum = np.float32(S1 - self.ns - self.OV)
        t_sum = np.float32(self.S2 - self.ns - self.OV2)
        return np.asarray(np.float32(c_sum / t_sum))

    def combine_device(self, results):
        S1 = float(self.Rsum)
        for i, (kind, r, x) in enumerate(self.ops):
            ln = self.lane[i]
            if ln == "pe":
                continue
            L = self.rows[r]["wh"] if kind == "head" else x[1] - x[0]
            sgn = 1.0 if self.rows[r]["suffix"] else -1.0
            for k in range(NCORES):
                a = results[k][f"o_acc_{ln}"][:, i].astype(np.float64)
                if ln == "act":
                    cntk = (L * P + a.sum()) / 2.0
                else:
                    cntk = a.sum()
                S1 += sgn * cntk
        for k in range(NCORES):
            ps = results[k]["o_ps"].astype(np.float64).reshape(-1)
            S1 += ps[:MM].sum() - ps[MM:].sum()
        return self._final(S1)


def build_bass(plan):
    nc = bacc.Bacc(debug=False, num_devices=NCORES)
    nrow, nops = plan.nrow, len(plan.ops)
    HW = plan.HW

    ybc = nc.dram_tensor("ybc", [1, N], FP16, kind="ExternalInput")
    hg = nc.dram_tensor("hg", [P, HW], FP16, kind="ExternalInput")
    cols = nc.dram_tensor("cols", [P, nops], FP32, kind="ExternalInput")
    o_accs = {ln: nc.dram_tensor(f"o_acc_{ln}", [P, nops], FP32,
                                 kind="ExternalOutput")
              for ln in ("dve", "act")}
    o_ps = nc.dram_tensor("o_ps", [1, 2 * MM], FP32, kind="ExternalOutput")

    n_mm = [0, 0]
    for i, (kind, r, x) in enumerate(plan.ops):
        if plan.lane[i] == "pe":
            L = plan.rows[r]["wh"] if kind == "head" else x[1] - x[0]
            n_mm[0 if plan.rows[r]["suffix"] else 1] += L // MM

    rings = {0: nc.sync, 1: nc.gpsimd, 2: nc.scalar}

    with tile.TileContext(nc) as tc:
        with (
            tc.tile_pool(name="c", bufs=1) as cpool,
            tc.tile_pool(name="ps", bufs=1, space="PSUM") as ppool,
        ):
            colt = cpool.tile([P, nops], FP32)
            hgt = cpool.tile([P, HW], FP16)
            bts = {}
            for (knd, val) in plan.dma_items:
                rg = rings[plan.dma_ring[(knd, val)]]
                if knd == "cols":
                    rg.dma_start(out=colt[:, :], in_=cols[:, :])
                elif knd == "head":
                    h0, h1 = int(plan.hoff[val]), int(plan.hoff[val + 1])
                    rg.dma_start(out=hgt[:, h0:h1], in_=hg[:, h0:h1])
                else:
                    cs, ce = val
                    t = cpool.tile([P, ce - cs], FP16, name=f"b{cs}")
                    rg.dma_start(
                        out=t[:, :],
                        in_=ybc[0:1, cs:ce].to_broadcast((P, ce - cs)))
                    bts[(cs, ce)] = t

            def locate(s0, e0):
                for (bs, be), t in bts.items():
                    if s0 >= bs and e0 <= be:
                        return t[:, s0 - bs:e0 - bs]
                raise KeyError((s0, e0))

            accs = {ln: cpool.tile([P, nops], FP32, name=f"acc_{ln}")
                    for ln in ("dve", "act")}
            SW = max([TCH] + [row["wh"] for row in plan.rows])
            scratch = {ln: cpool.tile([P, SW], FP16, name=f"s_{ln}")
                       for ln in ("dve", "act")}
            zscr = [cpool.tile([P, SW], BF16, name=f"z{j}") for j in range(3)]
            ones_w = cpool.tile([P, 1], BF16)
            nc.vector.memset(ones_w[:, :], 1.0)
            banks = [ppool.tile([1, MM], FP32, name="bankS"),
                     ppool.tile([1, MM], FP32, name="bankP")]
            seen = [0, 0]
            zrot = [0]

            def emit(i):
                kind, r, x = plan.ops[i]
                ln = plan.lane[i]
                if kind == "head":
                    h0 = int(plan.hoff[r])
                    L = plan.rows[r]["wh"]
                    src = hgt[:, h0:h0 + L]
                else:
                    src = locate(x[0], x[1])
                    L = x[1] - x[0]
                if ln == "dve":
                    nc.vector.tensor_scalar(
                        out=scratch[ln][:, 0:L], in0=src,
                        scalar1=colt[:, i:i + 1], scalar2=0.0,
                        op0=Alu.is_ge, op1=Alu.add,
                        accum_out=accs[ln][:, i:i + 1])
                elif ln == "act":
                    nc.scalar.activation(
                        out=scratch[ln][:, 0:L], in_=src, func=ActF.Sign,
                        bias=colt[:, i:i + 1], scale=1.0,
                        accum_out=accs[ln][:, i:i + 1])
                else:
                    b = 0 if plan.rows[r]["suffix"] else 1
                    z = zscr[zrot[0] % 3]
                    zrot[0] += 1
                    nc.vector.tensor_scalar(
                        out=z[:, 0:L], in0=src,
                        scalar1=colt[:, i:i + 1], scalar2=None,
                        op0=Alu.is_ge)
                    for ch in range(L // MM):
                        seen[b] += 1
                        nc.tensor.matmul(
                            banks[b][0:1, 0:MM], ones_w[:, :],
                            z[:, ch * MM:(ch + 1) * MM],
                            start=(seen[b] == 1),
                            stop=(seen[b] == n_mm[b]))

            for i in plan.emit_order:
                emit(i)

            stg = cpool.tile([1, 2 * MM], FP32)
            for b in range(2):
                if n_mm[b] == 0:
                    nc.vector.memset(stg[:, b * MM:(b + 1) * MM], 0.0)
                else:
                    nc.vector.tensor_copy(out=stg[:, b * MM:(b + 1) * MM],
                                          in_=banks[b][0:1, :])
            rings[0].dma_start(out=o_ps[:, :], in_=stg[:, :])
            for ln in ("dve", "act"):
                rings[0].dma_start(out=o_accs[ln][:, :], in_=accs[ln][:, :])

    nc.compile()
    return nc


_NC_CACHE = {}


def _get_nc(plan):
    key = plan.pattern
    if key not in _NC_CACHE:
        _NC_CACHE[key] = build_bass(plan)
    return _NC_CACHE[key]


def kernel(y, y_hat, status, _run_kwargs=None, _simulate=False):
    plan = Plan(y, y_hat, status)
    if _simulate:
        acc = plan.simulate()
        return plan.combine_from_acc([acc[k] for k in range(NCORES)])
    nc = _get_nc(plan)
    ybc2 = np.ascontiguousarray(plan.yh16.reshape(1, N))
    in_maps = []
    for k in range(NCORES):
        in_maps.append({
            "ybc": ybc2,
            "hg": np.ascontiguousarray(plan.hg[k]),
            "cols": np.ascontiguousarray(
                np.stack([plan.op_cols[i][k] for i in range(len(plan.ops))],
                         axis=1)),
        })
    kw = dict(_run_kwargs or {})
    res = bass_utils.run_bass_kernel_spmd(
        nc, in_maps, core_ids=list(range(NCORES)), **kw)
    out = plan.combine_device(res.results)
    if _run_kwargs is not None:
        return out, res
    return out


if __name__ == "__main__":
    rng = np.random.default_rng(0)
    y = rng.standard_normal(N).astype(np.float32)
    yh = rng.standard_normal(N).astype(np.float32)
    st = (rng.integers(0, 2, N)).astype(np.int32)
    print(kernel(y, yh, st, _simulate=True))


# revision 22
# speedup vs baseline: 2.0513x; 1.0932x over previous
"""Concordance-index (C-index) kernel for Trainium2, 8 NeuronCores — v4.2.

Math
----
Reference computes, over all pairs i<j of N=16384 samples:
    cc = ((y_i>=y_j & yh_i>=yh_j & st_j) | (y_i<=y_j & yh_i<=yh_j & st_i)) & triu
    tp = ((y_i<=y_j & st_i) | (y_i>=y_j & st_j)) & triu
    out = sum(cc) / sum(tp)

With A_ae = [y_a >= y_e], B_ae = [yh_a >= yh_e] over (a = all, e = event):
    sum(cc) = S1 - ns - OV,   S1 = sum_e T_e,  T_e = sum_a A_ae * B_ae
    sum(tp) = S2 - ns - OV2,  S2 = sum_e sum_a A_ae
where OV / OV2 are exact-fp32-tie corrections among event pairs.

Key ideas
---------
* Sort the a-axis by fp32 y: A_ae = [a >= c_e], c_e host-exact, so S2 is
  pure host math and T_e is a 1-D count of [yh_a >= t_e] over the suffix
  [c_e, N) — or R_e (host-known total) minus the prefix count.  Each
  event pays only its SHORTER side: ~N^2/8 compares total.
* Events sorted by c, 1024 per row (128/core x 8 cores).  The ragged
  per-event boundary is absorbed into a HOST-GATHERED window per event
  (padded with -60000 -> compares contribute 0); the rest is a
  row-uniform 64-aligned tail.  Every device op is a plain fused
  compare+accumulate — no masks, no products.
* S1 only needs GLOBAL sums, so three lanes run concurrently:
  ACT (Sign+accum), DVE (is_ge+add fused), and DVE plain is_ge in 4x
  DVE perf mode feeding PE ones-matmul column reductions in PSUM.
* DMA is chunked over three queue rings (sync/gpsimd/tensor) in the
  order ops consume it; ops are emitted in estimated-arrival order.
* fp16 compare semantics host+device; only fp16-vs-fp32 tie noise
  remains (~1e-4 rel; gate 2e-2).
"""

import math
import os
import sys

import numpy as np

for _p in ("/opt/trn_rl_repo", "/root/.axon_site", "/root/.axon_site/_ro/trn_rl_repo"):
    if os.path.isdir(_p) and _p not in sys.path:
        sys.path.append(_p)

import ml_dtypes  # noqa: F401  (env check)

import concourse.bacc as bacc
import concourse.mybir as mybir
from concourse import bass_utils
from concourse import tile

N = 16384
P = 128
NCORES = 8
NEG16 = np.float16(-60000.0)
BIG32 = np.float32(1e30)
TCH = 3072          # tail/bcast chunk width
MM = 512            # PE matmul moving width (one PSUM bank)

FP32 = mybir.dt.float32
FP16 = mybir.dt.float16
BF16 = mybir.dt.bfloat16
Alu = mybir.AluOpType
ActF = mybir.ActivationFunctionType

# measured ns/elem and ns/op overhead (TRN2, aligned ops)
COST = {
    "dve": (1.09, 240.0),    # tensor_scalar is_ge+add fused accum (1x)
    "act": (0.95, 600.0),    # activation Sign with accum (big read-acc cost)
    "pe": (0.29, 130.0),     # DVE plain is_ge at 4x (feeds PE)
}
ACT_MIN = 2048               # ACT lane only for ops at least this wide
PE_RATE = 1.05               # ns/elem ones-matmul reduce incl ldweights
DMA_NAT = 6.1e-3             # ns/SBUF-byte, natural DMA (~163 GB/s, HBM-read bound)
DMA_BC = 1.9e-3              # ns/SBUF-byte, broadcast DMA (~530 GB/s)


def _mid32(t16):
    prev = np.nextafter(t16, np.float16(-np.inf))
    return (t16.astype(np.float32) + prev.astype(np.float32)) * np.float32(0.5)


class Plan:
    def __init__(self, y, yh, status):
        y32 = np.asarray(y, np.float32)
        yh32 = np.asarray(yh, np.float32)
        st = np.asarray(status)

        order = np.argsort(y32, kind="stable")
        ys = y32[order]
        self.yh16 = yh32[order].astype(np.float16)

        ev = np.nonzero(st == 1)[0]
        self.ns = int(len(ev))
        t16 = yh32[ev].astype(np.float16)
        c = np.searchsorted(ys, y32[ev], side="left").astype(np.int64)
        yh16s = np.sort(self.yh16)
        R = (N - np.searchsorted(yh16s, t16, side="left")).astype(np.int64)
        self.S2 = int((np.int64(N) - c).sum())

        yv = y32[ev]
        _, cnt = np.unique(yv, return_counts=True)
        self.OV2 = int((cnt.astype(np.int64) * (cnt - 1) // 2).sum())
        pair = np.stack([yv, yh32[ev]], 1)
        _, cnt2 = np.unique(pair, axis=0, return_counts=True)
        self.OV = int((cnt2.astype(np.int64) * (cnt2 - 1) // 2).sum())

        eo = np.argsort(c, kind="stable")
        self.c = c[eo]
        self.t16 = t16[eo]
        self.R = R[eo]

        self.nrow = max(1, math.ceil(self.ns / (P * NCORES)))
        self.rows = []
        for r in range(self.nrow):
            s, e = r * P * NCORES, min((r + 1) * P * NCORES, self.ns)
            cr = self.c[s:e]
            lo, hi = int(cr.min()), int(cr.max())
            suffix = int((np.int64(N) - cr).sum()) <= int(cr.sum())
            if suffix:
                a = min(N, ((hi + 1 + MM - 1) // MM) * MM)  # 512-aligned
                tail = (a, N)
                sel = [j for j in range((lo + MM - 1) // MM, a // MM)]
            else:
                b = (lo // MM) * MM
                tail = (0, b)
                sel = [j for j in range(b // MM, N // MM)
                       if MM * (j + 1) <= hi]
            self.rows.append({"suffix": suffix, "s": s, "e": e, "lo": lo,
                              "hi": hi, "wh": MM, "tail": tail, "sel": sel,
                              "abound": a if suffix else b})

        # ---- broadcast ranges: tails + selective blocks, merged ----
        cover = [row["tail"] for row in self.rows
                 if row["tail"][1] > row["tail"][0]]
        for row in self.rows:
            for j in row["sel"]:
                cover.append((j * MM, (j + 1) * MM))
        cover.sort()
        merged = []
        for s0, e0 in cover:
            if merged and s0 <= merged[-1][1]:
                merged[-1][1] = max(merged[-1][1], e0)
            else:
                merged.append([s0, e0])
        self.chunks = []        # (start, end) bcast tiles
        for s0, e0 in merged:
            x = s0
            while x < e0:
                self.chunks.append((x, min(x + TCH, e0)))
                x += TCH

        # ---- op list: partial-head + selective blocks + tail pieces ----
        self.ops = []           # (kind, row, (start, end) or None)
        for r, row in enumerate(self.rows):
            self.ops.append(("head", r, None))
            for j in row["sel"]:
                self.ops.append(("sel", r, (j * MM, (j + 1) * MM)))
            t0, t1 = row["tail"]
            for (cs, ce) in self.chunks:
                s0, e0 = max(t0, cs), min(t1, ce)
                if s0 < e0:
                    self.ops.append(("tail", r, (s0, e0)))

        def op_elems(i):
            kind, r, x = self.ops[i]
            return self.rows[r]["wh"] if kind == "head" else x[1] - x[0]

        self.op_elems = op_elems

        # ---- DMA schedule: interleave head segments and bcast chunks
        # round-robin over 3 rings (sync / gpsimd / tensor) ----
        self.hoff = np.concatenate(
            [[0], np.cumsum([row["wh"] for row in self.rows])]).astype(int)
        HW = int(self.hoff[-1])
        self.HW = HW
        dma_items = [("cols", None)]
        for r in range(self.nrow):
            dma_items.append(("head", r))
        for cc in self.chunks:
            dma_items.append(("chunk", cc))
        ring_t = [0.0, 0.0, 0.0]
        self.dma_ring = {}
        self.head_arr = [0.0] * self.nrow
        self.chunk_arr = {}
        nch = 0
        for (knd, val) in dma_items:
            if knd == "cols":
                rg, sz, rate = 0, len(self.ops) * P * 4, DMA_NAT
            elif knd == "head":
                rg, sz, rate = 0, int(self.rows[val]["wh"]) * P * 2, DMA_NAT
            else:
                # broadcast chunks alternate ACT ring (fast, idle early)
                # and SP ring (after the hg segments)
                rg = 2 if nch % 2 == 0 else 0
                nch += 1
                sz, rate = (val[1] - val[0]) * P * 2, DMA_BC
            ring_t[rg] += sz * rate
            self.dma_ring[(knd, val)] = rg
            if knd == "head":
                self.head_arr[val] = ring_t[rg]
            elif knd == "chunk":
                self.chunk_arr[val] = ring_t[rg]
        self.dma_items = dma_items

        def op_arrival(i):
            kind, r, x = self.ops[i]
            if kind == "head":
                return self.head_arr[r]
            for (cs, ce) in self.chunks:
                if x[0] >= cs and x[1] <= ce:
                    return self.chunk_arr[(cs, ce)]
            raise KeyError((kind, r, x))

        # ---- lane assignment: quota split (LP balance), arrival order ----
        E = float(sum(op_elems(i) for i in range(len(self.ops))))
        rd, ra, rz = COST["dve"][0], COST["act"][0], COST["pe"][0]
        # act*x = dve: rd*y + rz*z = pe: PE_RATE*z = T ; x+y+z = E
        T = E / (1.0 / ra + (1.0 - rz / PE_RATE) / rd + 1.0 / PE_RATE)
        quota = {"act": T / ra, "pe": T / PE_RATE}
        quota["dve"] = E - quota["act"] - quota["pe"]
        order_i = sorted(range(len(self.ops)), key=op_arrival)
        used = {"dve": 0.0, "act": 0.0, "pe": 0.0}
        load = {"dve": 0.0, "act": 0.0, "pe": 0.0}
        self.lane = [None] * len(self.ops)
        for i in order_i:
            el = op_elems(i)
            arr = op_arrival(i)
            cands = ["dve"] + (["act"] if el >= ACT_MIN else []) \
                + (["pe"] if el % MM == 0 else [])
            best = max(cands, key=lambda ln: quota[ln] - used[ln])
            self.lane[i] = best
            used[best] += el
            r0, o0 = COST[best]
            if best == "pe":
                t0 = max(load["dve"], arr) + el * r0 + o0
                load["dve"] = t0
                load["pe"] = max(load["pe"], t0) + el * PE_RATE
            else:
                load[best] = max(load[best], arr) + el * r0 + o0
        self.load = load
        self.emit_order = order_i

        self._build_core_data()
        self.pattern = (self.nrow,
                        tuple((row["suffix"], row["wh"], row["tail"],
                               tuple(row["sel"])) for row in self.rows),
                        tuple(self.lane), tuple(self.chunks))

    def _build_core_data(self):
        nrow = self.nrow
        HW = self.HW
        self.slot_ev = np.full((NCORES, nrow, P), -1, np.int64)
        self.cslot = np.full((NCORES, nrow, P), -1, np.int64)
        self.hg = np.full((NCORES, P, HW), NEG16, np.float16)
        self.thr = np.full((NCORES, nrow, P), BIG32, np.float32)
        self.nmid = np.full((NCORES, nrow, P), -BIG32, np.float32)
        mids = _mid32(self.t16)
        self.Rsum = 0
        for r, row in enumerate(self.rows):
            s, e = row["s"], row["e"]
            idx = np.arange(s, e)
            if not row["suffix"]:
                self.Rsum += int(self.R[s:e].sum())
            for k in range(NCORES):
                mine = idx[k::NCORES]
                self.slot_ev[k, r, :len(mine)] = mine
                ce = self.c[mine]
                self.cslot[k, r, :len(mine)] = ce
                self.thr[k, r, :len(mine)] = self.t16[mine].astype(np.float32)
                self.nmid[k, r, :len(mine)] = -mids[mine]
                h0 = int(self.hoff[r])
                if row["suffix"]:
                    # partial head [c_e, MM*ceil(c_e/MM))
                    for p, cc in enumerate(ce):
                        a = int(-(-int(cc) // MM)) * MM
                        L = min(a, N) - int(cc)
                        if L > 0:
                            self.hg[k, p, h0:h0 + L] = \
                                self.yh16[int(cc):int(cc) + L]
                else:
                    # partial head [MM*floor(c_e/MM), c_e)
                    for p, cc in enumerate(ce):
                        b = (int(cc) // MM) * MM
                        L = int(cc) - b
                        if L > 0:
                            self.hg[k, p, h0:h0 + L] = \
                                self.yh16[b:int(cc)]

        self.op_cols = []
        for i, (kind, r, x) in enumerate(self.ops):
            ln = self.lane[i]
            base = self.nmid[:, r, :] if ln == "act" else self.thr[:, r, :]
            if kind == "sel":
                j = x[0] // MM
                cs = self.cslot[:, r, :]
                if self.rows[r]["suffix"]:
                    active = (cs >= 0) & (cs <= j * MM)
                else:
                    active = (cs >= 0) & (cs >= (j + 1) * MM)
                maskv = np.float32(-BIG32) if ln == "act" else BIG32
                col = np.where(active, base, maskv).astype(np.float32)
            else:
                col = base.astype(np.float32)
            self.op_cols.append(col)

    # ---- numpy simulation ----
    def simulate(self):
        nops = len(self.ops)
        acc = np.zeros((NCORES, P, nops), np.float64)
        for i, (kind, r, x) in enumerate(self.ops):
            ln = self.lane[i]
            h0 = int(self.hoff[r])
            for k in range(NCORES):
                if kind == "head":
                    data = self.hg[k][:, h0:h0 + self.rows[r]["wh"]]
                    data = data.astype(np.float32)
                else:
                    s0, e0 = x
                    data = np.broadcast_to(
                        self.yh16[s0:e0].astype(np.float32), (P, e0 - s0))
                col = self.op_cols[i][k][:, None]
                if ln == "act":
                    sg = np.where(data > -col, 1.0, -1.0)
                    acc[k, :, i] = sg.sum(axis=1)
                else:
                    acc[k, :, i] = (data >= col).sum(axis=1)
        return acc

    def combine_from_acc(self, accs):
        S1 = float(self.Rsum)
        for i, (kind, r, x) in enumerate(self.ops):
            L = self.rows[r]["wh"] if kind == "head" else x[1] - x[0]
            sgn = 1.0 if self.rows[r]["suffix"] else -1.0
            for k in range(NCORES):
                a = accs[k][:, i].astype(np.float64)
                if self.lane[i] == "act":
                    cntk = (L * P + a.sum()) / 2.0
                else:
                    cntk = a.sum()
                S1 += sgn * cntk
        return self._final(S1)

    def _final(self, S1):
        c_sum = np.float32(S1 - self.ns - self.OV)
        t_sum = np.float32(self.S2 - self.ns - self.OV2)
        return np.asarray(np.float32(c_sum / t_sum))

    def combine_device(self, results):
        S1 = float(self.Rsum)
        for i, (kind, r, x) in enumerate(self.ops):
            ln = self.lane[i]
            if ln == "pe":
                continue
            L = self.rows[r]["wh"] if kind == "head" else x[1] - x[0]
            sgn = 1.0 if self.rows[r]["suffix"] else -1.0
            for k in range(NCORES):
                a = results[k][f"o_acc_{ln}"][:, i].astype(np.float64)
                if ln == "act":
                    cntk = (L * P + a.sum()) / 2.0
                else:
                    cntk = a.sum()
                S1 += sgn * cntk
        for k in range(NCORES):
            ps = results[k]["o_ps"].astype(np.float64).reshape(-1)
            S1 += ps[:MM].sum() - ps[MM:].sum()
        return self._final(S1)


def build_bass(plan):
    nc = bacc.Bacc(debug=False, num_devices=NCORES)
    nrow, nops = plan.nrow, len(plan.ops)
    HW = plan.HW

    ybc = nc.dram_tensor("ybc", [1, N], FP16, kind="ExternalInput")
    hg = nc.dram_tensor("hg", [P, HW], FP16, kind="ExternalInput")
    cols = nc.dram_tensor("cols", [P, nops], FP32, kind="ExternalInput")
    o_accs = {ln: nc.dram_tensor(f"o_acc_{ln}", [P, nops], FP32,
                                 kind="ExternalOutput")
              for ln in ("dve", "act")}
    o_ps = nc.dram_tensor("o_ps", [1, 2 * MM], FP32, kind="ExternalOutput")

    n_mm = [0, 0]
    for i, (kind, r, x) in enumerate(plan.ops):
        if plan.lane[i] == "pe":
            L = plan.rows[r]["wh"] if kind == "head" else x[1] - x[0]
            n_mm[0 if plan.rows[r]["suffix"] else 1] += L // MM

    rings = {0: nc.sync, 1: nc.gpsimd, 2: nc.scalar}

    with tile.TileContext(nc) as tc:
        with (
            tc.tile_pool(name="c", bufs=1) as cpool,
            tc.tile_pool(name="ps", bufs=1, space="PSUM") as ppool,
        ):
            colt = cpool.tile([P, nops], FP32)
            hgt = cpool.tile([P, HW], FP16)
            bts = {}
            for (knd, val) in plan.dma_items:
                rg = rings[plan.dma_ring[(knd, val)]]
                if knd == "cols":
                    rg.dma_start(out=colt[:, :], in_=cols[:, :])
                elif knd == "head":
                    h0, h1 = int(plan.hoff[val]), int(plan.hoff[val + 1])
                    rg.dma_start(out=hgt[:, h0:h1], in_=hg[:, h0:h1])
                else:
                    cs, ce = val
                    t = cpool.tile([P, ce - cs], FP16, name=f"b{cs}")
                    rg.dma_start(
                        out=t[:, :],
                        in_=ybc[0:1, cs:ce].to_broadcast((P, ce - cs)))
                    bts[(cs, ce)] = t

            def locate(s0, e0):
                for (bs, be), t in bts.items():
                    if s0 >= bs and e0 <= be:
                        return t[:, s0 - bs:e0 - bs]
                raise KeyError((s0, e0))

            accs = {ln: cpool.tile([P, nops], FP32, name=f"acc_{ln}")
                    for ln in ("dve", "act")}
            SW = max([TCH] + [row["wh"] for row in plan.rows])
            scratch = {ln: cpool.tile([P, SW], FP16, name=f"s_{ln}")
                       for ln in ("dve", "act")}
            zscr = [cpool.tile([P, SW], BF16, name=f"z{j}") for j in range(3)]
            ones_w = cpool.tile([P, 1], BF16)
            nc.vector.memset(ones_w[:, :], 1.0)
            banks = [ppool.tile([1, MM], FP32, name="bankS"),
                     ppool.tile([1, MM], FP32, name="bankP")]
            seen = [0, 0]
            zrot = [0]

            def emit(i):
                kind, r, x = plan.ops[i]
                ln = plan.lane[i]
                if kind == "head":
                    h0 = int(plan.hoff[r])
                    L = plan.rows[r]["wh"]
                    src = hgt[:, h0:h0 + L]
                else:
                    src = locate(x[0], x[1])
                    L = x[1] - x[0]
                if ln == "dve":
                    nc.vector.tensor_scalar(
                        out=scratch[ln][:, 0:L], in0=src,
                        scalar1=colt[:, i:i + 1], scalar2=0.0,
                        op0=Alu.is_ge, op1=Alu.add,
                        accum_out=accs[ln][:, i:i + 1])
                elif ln == "act":
                    nc.scalar.activation(
                        out=scratch[ln][:, 0:L], in_=src, func=ActF.Sign,
                        bias=colt[:, i:i + 1], scale=1.0,
                        accum_out=accs[ln][:, i:i + 1])
                else:
                    b = 0 if plan.rows[r]["suffix"] else 1
                    z = zscr[zrot[0] % 3]
                    zrot[0] += 1
                    nc.vector.tensor_scalar(
                        out=z[:, 0:L], in0=src,
                        scalar1=colt[:, i:i + 1], scalar2=None,
                        op0=Alu.is_ge)
                    for ch in range(L // MM):
                        seen[b] += 1
                        nc.tensor.matmul(
                            banks[b][0:1, 0:MM], ones_w[:, :],
                            z[:, ch * MM:(ch + 1) * MM],
                            start=(seen[b] == 1),
                            stop=(seen[b] == n_mm[b]))

            for i in plan.emit_order:
                emit(i)

            stg = cpool.tile([1, 2 * MM], FP32)
            for b in range(2):
                if n_mm[b] == 0:
                    nc.vector.memset(stg[:, b * MM:(b + 1) * MM], 0.0)
                else:
                    nc.vector.tensor_copy(out=stg[:, b * MM:(b + 1) * MM],
                                          in_=banks[b][0:1, :])
            rings[0].dma_start(out=o_ps[:, :], in_=stg[:, :])
            for ln in ("dve", "act"):
                rings[0].dma_start(out=o_accs[ln][:, :], in_=accs[ln][:, :])

    nc.compile()
    return nc


_NC_CACHE = {}


def _get_nc(plan):
    key = plan.pattern
    if key not in _NC_CACHE:
        _NC_CACHE[key] = build_bass(plan)
    return _NC_CACHE[key]


def kernel(y, y_hat, status, _run_kwargs=None, _simulate=False):
    plan = Plan(y, y_hat, status)
    if _simulate:
        acc = plan.simulate()
        return plan.combine_from_acc([acc[k] for k in range(NCORES)])
    nc = _get_nc(plan)
    ybc2 = np.ascontiguousarray(plan.yh16.reshape(1, N))
    in_maps = []
    for k in range(NCORES):
        in_maps.append({
            "ybc": ybc2,
            "hg": np.ascontiguousarray(plan.hg[k]),
            "cols": np.ascontiguousarray(
                np.stack([plan.op_cols[i][k] for i in range(len(plan.ops))],
                         axis=1)),
        })
    kw = dict(_run_kwargs or {})
    res = bass_utils.run_bass_kernel_spmd(
        nc, in_maps, core_ids=list(range(NCORES)), **kw)
    out = plan.combine_device(res.results)
    if _run_kwargs is not None:
        return out, res
    return out


if __name__ == "__main__":
    rng = np.random.default_rng(0)
    y = rng.standard_normal(N).astype(np.float32)
    yh = rng.standard_normal(N).astype(np.float32)
    st = (rng.integers(0, 2, N)).astype(np.int32)
    print(kernel(y, yh, st, _simulate=True))
